# revision 16
# baseline (speedup 1.0000x reference)
"""GCNGuard forward on 8 Trainium2 NeuronCores (Bass/Tile) — fast path.

Key observation: with drop_W = [w0 > 0, w1 >= 0] and drop_b = 0 the learned
drop gate is a no-op (z = att*w0 + att_rev*w1 >= 0, and z > 0 whenever
att > 0, so att * (sigmoid(z) > .5) == att).  The whole reverse-edge /
rs-AllGather machinery of the general path is dead: each layer is a single
fused pass needing only row-local quantities plus gathered [hn | s] rows of
the edge's col endpoint.

Layout: nodes sharded 6250/core, degree-sorted into 49 windows of 128 rows.
Edges live at (window, column j, partition r) = row-major slots, so the
one-hot scatter matrix is the identity: rs/deg are free-axis reduces, and
agg^T accumulates on the PE as sum_c matmul(lhsT=s_gathered[c], rhs=
diag(w[:, c])).  Per layer each core publishes its [hn16 | s16] rows into a
shared table via NCHUNK AllGathers (chunk-major layout keeps each collective
contiguous) overlapped behind the edge pass; edges gather 1 KiB fp16 pairs
(pair id fits int16).  Sims: per-column stt with the pair-select mask folded
into the per-partition scalar (simL on DVE, simR on GpSimd); diag builds
split DVE/Act.  fp16 tables measured at 5.9e-4 end-to-end rel err.
"""

import os
from contextlib import ExitStack

import numpy as np

P = 128
D = 128
NC = 8
EPS = 1e-5

# ---------------------------------------------------------------------------
# host-side preprocessing (fast path)
# ---------------------------------------------------------------------------


def _pack_idx16(flat):
    """[n] int64 row ids -> [128, n//16] int16 dma_gather index layout."""
    n = flat.shape[0]
    assert n % 128 == 0
    out = np.zeros((P, n // 16), np.uint16)
    cols = np.arange(n) // 16
    rows = np.arange(n) % 16
    for g in range(8):
        out[g * 16 + rows, cols] = flat.astype(np.uint16)
    return out.view(np.int16)


def _prep_fast(row, col, n):
    row = np.asarray(row).astype(np.int64)
    col = np.asarray(col).astype(np.int64)
    E = row.shape[0]
    NPC = -(-n // NC)
    R = -(-NPC // P) * P
    W = R // P
    NCHUNK = int(os.environ.get("GG_NCHUNK", "3"))
    NCHUNK = max(1, min(NCHUNK, W))
    CHB = [round(i * W / NCHUNK) * P for i in range(NCHUNK + 1)]  # row bounds

    deg = np.bincount(row, minlength=n)
    perm_pos = np.empty(n, np.int64)
    node_core = np.minimum(np.arange(n) // NPC, NC - 1)
    orders = []
    CWc = np.zeros((NC, W), np.int64)
    for c in range(NC):
        lo, hi = c * NPC, min((c + 1) * NPC, n)
        dd = deg[lo:hi]
        order = np.argsort(-dd, kind="stable")
        pos = np.empty(hi - lo, np.int64)
        pos[order] = np.arange(hi - lo)
        perm_pos[lo:hi] = pos
        orders.append(order + lo)
        sd = np.concatenate([dd[order], np.zeros(R - (hi - lo), np.int64)])
        CWc[c] = sd.reshape(W, P).max(axis=1)
    CW = CWc.max(axis=0).astype(np.int64)
    OWS = np.zeros(W + 1, np.int64)
    OWS[1:] = np.cumsum(CW)
    SC = int(OWS[-1])

    # chunk-major table row: rows of chunk k from all cores are contiguous
    pp = perm_pos
    chb = np.asarray(CHB, np.int64)
    kk = np.searchsorted(chb, pp, side="right") - 1
    chr_k = chb[kk + 1] - chb[kk]                    # rows in node's chunk
    tabrow = NC * chb[kk] + node_core * chr_k + (pp - chb[kk])

    ecore = np.minimum(row // NPC, NC - 1)
    lr = perm_pos[row]
    wv = lr // P
    rv = lr % P
    okey = ecore * R + lr
    eorder = np.argsort(okey, kind="stable")
    sk = okey[eorder]
    starts = np.r_[0, np.flatnonzero(np.diff(sk)) + 1]
    grp = np.zeros(E, np.int64)
    grp[starts] = 1
    grp = np.cumsum(grp) - 1
    jj = np.arange(E) - starts[grp]

    ec = ecore[eorder]
    flat = (OWS[wv[eorder]] + jj) * P + rv[eorder]
    tr = tabrow[col[eorder]]

    idx_all = np.zeros((NC, SC * P), np.int64)
    mL_all = np.zeros((NC, P, SC), np.float32)
    mR_all = np.zeros((NC, P, SC), np.float32)
    idx_all[ec, flat] = tr // 2
    left = (tr % 2 == 0).astype(np.float32)
    cidx = flat // P
    ridx = flat % P
    mL_all[ec, ridx, cidx] = left
    mR_all[ec, ridx, cidx] = 1.0 - left

    idx16 = [_pack_idx16(idx_all[c]) for c in range(NC)]

    return dict(
        R=R, W=W, NPC=NPC, NCHUNK=NCHUNK, CHB=tuple(CHB),
        CW=tuple(int(v) for v in CW), OWS=OWS, SC=SC,
        idx16=idx16, mL=[np.ascontiguousarray(mL_all[c]) for c in range(NC)],
        mR=[np.ascontiguousarray(mR_all[c]) for c in range(NC)],
        orders=orders,
    )


# ---------------------------------------------------------------------------
# bass program (fast path)
# ---------------------------------------------------------------------------


def _build_fast(R, W, CW, CHB, ln_trivial, b_zero,
                gsplit=64):
    import concourse.bass as bass  # noqa: F401
    import concourse.bacc as bacc
    import concourse.mybir as mybir
    import concourse.tile as tile
    from concourse.masks import make_identity

    F32 = mybir.dt.float32
    F16 = mybir.dt.float16
    I16 = mybir.dt.int16
    AF = mybir.ActivationFunctionType
    OP = mybir.AluOpType

    NPAD = NC * R
    OWS = [0]
    for c in CW:
        OWS.append(OWS[-1] + c)
    SC = OWS[-1]
    CWmax = max(CW)
    RG = [list(range(NC))]

    nc = bacc.Bacc("TRN2", target_bir_lowering=False)

    x_in = nc.dram_tensor("x", [R, D], F32, kind="ExternalInput")
    w0_in = nc.dram_tensor("W0", [D, D], F32, kind="ExternalInput")
    w1_in = nc.dram_tensor("W1", [D, D], F32, kind="ExternalInput")
    b0_in = nc.dram_tensor("b0", [1, D], F32, kind="ExternalInput")
    b1_in = nc.dram_tensor("b1", [1, D], F32, kind="ExternalInput")
    idx_in = nc.dram_tensor("idx16", [P, SC * 8], I16, kind="ExternalInput")
    ml_in = nc.dram_tensor("mL", [P, SC], F32, kind="ExternalInput")
    mr_in = nc.dram_tensor("mR", [P, SC], F32, kind="ExternalInput")
    lng_in = nc.dram_tensor("lng", [2, D], F32, kind="ExternalInput")
    lnb_in = nc.dram_tensor("lnb", [2, D], F32, kind="ExternalInput")
    out_t = nc.dram_tensor("out", [R, D], F32, kind="ExternalOutput")

    TAB = [nc.dram_tensor(f"tab{i}", [NPAD, D], F32, kind="Internal",
                          addr_space="Shared") for i in range(2)]
    con = [nc.dram_tensor(f"con{i}", [R, D], F32, kind="Internal")
           for i in range(2)]

    with tile.TileContext(nc) as tc, ExitStack() as ctx:
        singles = ctx.enter_context(tc.tile_pool(name="singles", bufs=1))
        gpool = ctx.enter_context(tc.tile_pool(name="gpool", bufs=4))
        spool = ctx.enter_context(tc.tile_pool(name="spool", bufs=3))
        wpool = ctx.enter_context(tc.tile_pool(name="wpool", bufs=3))
        dpool = ctx.enter_context(tc.tile_pool(name="dpool", bufs=6))
        hpool = ctx.enter_context(tc.tile_pool(name="hpool", bufs=3))
        psTR = ctx.enter_context(tc.tile_pool(name="psTR", bufs=2, space="PSUM"))
        psS = ctx.enter_context(tc.tile_pool(name="psS", bufs=2, space="PSUM"))
        psAG = ctx.enter_context(tc.tile_pool(name="psAG", bufs=2, space="PSUM"))
        psA2 = ctx.enter_context(tc.tile_pool(name="psA2", bufs=2, space="PSUM"))

        ident = singles.tile([P, P], F32)
        make_identity(nc, ident[:])
        ident16 = singles.tile([P, P], F16)
        nc.vector.tensor_copy(ident16[:], ident[:])

        _consts = {}

        def constcol(val):
            if val not in _consts:
                t = singles.tile([P, 1], F32, tag=f"const{len(_consts)}")
                nc.vector.memset(t[:], float(val))
                _consts[val] = t
            return _consts[val][:]

        w16 = []
        for t_in in (w0_in, w1_in):
            t32 = spool.tile([D, D], F32, tag="wld")
            nc.sync.dma_start(t32[:], t_in[:, :])
            t16 = singles.tile([D, D], F16, tag=f"w16{len(w16)}")
            nc.vector.tensor_copy(t16[:], t32[:])
            w16.append(t16)
        b_sb = [None, None]
        if not b_zero:
            for i, t_in in enumerate((b0_in, b1_in)):
                t = singles.tile([P, D], F32, tag=f"b{i}")
                nc.gpsimd.dma_start(t[:], t_in[0:1, :].to_broadcast([P, D]))
                b_sb[i] = t
        lng_sb = [None, None]
        lnb_sb = [None, None]
        if not ln_trivial:
            for i in range(2):
                g = singles.tile([P, D], F32, tag=f"lng{i}")
                nc.gpsimd.dma_start(g[:], lng_in[i:i + 1, :].to_broadcast([P, D]))
                lng_sb[i] = g
                b = singles.tile([P, D], F32, tag=f"lnb{i}")
                nc.gpsimd.dma_start(b[:], lnb_in[i:i + 1, :].to_broadcast([P, D]))
                lnb_sb[i] = b

        idx_sb = singles.tile([P, SC * 8], I16)
        nc.sync.dma_start(idx_sb[:], idx_in[:, :])
        mL_sb = singles.tile([P, SC], F32)
        nc.sync.dma_start(mL_sb[:], ml_in[:, :])
        mR_sb = singles.tile([P, SC], F32)
        nc.sync.dma_start(mR_sb[:], mr_in[:, :])

        def node_emit(h_sb, w, lnext):
            """h_sb: [P, D] f32 post-activation rows of window w.
            Publish hn16/s16 to con[lnext%2], s f32 to sscr[lnext%2]."""
            dst = con[lnext % 2]
            ss = wpool.tile([P, 1], F32, tag="nss")
            scr = spool.tile([P, D], F32, tag="nscr")
            nc.vector.scalar_tensor_tensor(
                out=scr[:], in0=h_sb[:], scalar=1.0, in1=h_sb[:],
                op0=OP.mult, op1=OP.mult, accum_out=ss[:])
            nc.scalar.activation(out=ss[:], in_=ss[:], func=AF.Sqrt,
                                 bias=constcol(1e-30))
            nc.vector.reciprocal(ss[:], ss[:])
            hn16 = spool.tile([P, D], F16, tag="hn16")
            nc.vector.tensor_scalar_mul(hn16[:], h_sb[:], ss[:])
            nc.sync.dma_start(
                dst[w * P:(w + 1) * P, :D // 2].bitcast(F16), hn16[:])
            hT_ps = psTR.tile([P, P], F32, tag="tr")
            nc.tensor.transpose(out=hT_ps[:], in_=h_sb[:], identity=ident[:])
            hT16 = spool.tile([P, D], F16, tag="hT16")
            nc.scalar.copy(hT16[:], hT_ps[:])
            s_ps = psS.tile([P, P], F32, tag="sps")
            nc.tensor.matmul(out=s_ps[:], lhsT=hT16[:],
                             rhs=w16[0 if lnext == 0 else 1][:],
                             start=True, stop=True)
            s16 = spool.tile([P, D], F16, tag="s16")
            nc.scalar.copy(s16[:], s_ps[:])
            nc.sync.dma_start(
                dst[w * P:(w + 1) * P, D // 2:D].bitcast(F16), s16[:])

        chend = {CHB[k + 1]: k for k in range(len(CHB) - 1)}

        def emit_chunk_cc(layer, w):
            k = chend.get((w + 1) * P)
            if k is None:
                return
            li = layer % 2
            lo, hi = CHB[k], CHB[k + 1]
            nc.gpsimd.collective_compute(
                "AllGather", OP.bypass, replica_groups=RG,
                ins=[con[li][lo:hi, :]],
                outs=[TAB[li][NC * lo:NC * hi, :]])

        # layer 0 node pass
        for w in range(W):
            x_sb = hpool.tile([P, D], F32, tag="x0")
            nc.sync.dma_start(x_sb[:], x_in[w * P:(w + 1) * P, :])
            node_emit(x_sb, w, 0)
            emit_chunk_cc(0, w)

        for layer in range(3):
            li = layer % 2
            tabv = TAB[li][:, :].rearrange("(a b) d -> a (b d)", b=2)
            for w in range(W):
                C = CW[w]
                ow = OWS[w]
                hs = spool.tile([P, 2 * D], F16, tag="hs")
                nc.sync.dma_start(hs[:],
                                  con[li][w * P:(w + 1) * P, :].bitcast(F16))
                sloc = hs[:, D:2 * D]
                if C == 0:
                    h2 = hpool.tile([P, D], F32, tag="h2")
                    nc.vector.tensor_scalar_mul(h2[:], sloc,
                                                float(np.exp(1.0)))
                else:
                    sC32 = gpool.tile([P, CWmax, 2 * D], F32, tag="sC")
                    for t0 in range(0, C, gsplit):
                        t1 = min(t0 + gsplit, C)
                        nc.gpsimd.dma_gather(
                            out_ap=sC32[:, t0:t1, :],
                            in_ap=tabv,
                            idxs_ap=idx_sb[:, (ow + t0) * 8:(ow + t1) * 8],
                            num_idxs=(t1 - t0) * P,
                            num_idxs_reg=(t1 - t0) * P,
                            elem_size=2 * D)
                    sC = sC32[:, :, :].bitcast(F16)

                    simL = wpool.tile([P, CWmax], F32, tag="simL")
                    simR = wpool.tile([P, CWmax], F32, tag="simR")
                    scrA = spool.tile([P, D], F16, tag="scrA")
                    scrB = spool.tile([P, D], F16, tag="scrB")
                    for c in range(C):
                        nc.vector.scalar_tensor_tensor(
                            out=scrA[:], in0=sC[:, c, 0:D],
                            scalar=mL_sb[:, ow + c:ow + c + 1], in1=hs[:, 0:D],
                            op0=OP.mult, op1=OP.mult,
                            accum_out=simL[:, c:c + 1])
                        nc.vector.scalar_tensor_tensor(
                            out=scrB[:], in0=sC[:, c, 2 * D:3 * D],
                            scalar=mR_sb[:, ow + c:ow + c + 1], in1=hs[:, 0:D],
                            op0=OP.mult, op1=OP.mult,
                            accum_out=simR[:, c:c + 1])
                    sims = wpool.tile([P, CWmax], F32, tag="sims")
                    nc.vector.tensor_tensor(out=sims[:, :C], in0=simL[:, :C],
                                            in1=simR[:, :C], op=OP.add)
                    thr = wpool.tile([P, CWmax], F32, tag="thr")
                    nc.vector.tensor_scalar(out=thr[:, :C], in0=sims[:, :C],
                                            scalar1=0.1, scalar2=None,
                                            op0=OP.is_ge)
                    nc.vector.tensor_tensor(out=sims[:, :C], in0=sims[:, :C],
                                            in1=thr[:, :C], op=OP.mult)
                    rs = wpool.tile([P, 1], F32, tag="rs")
                    nc.vector.tensor_reduce(out=rs[:], in_=sims[:, :C],
                                            axis=mybir.AxisListType.X,
                                            op=OP.add)
                    deg = wpool.tile([P, 1], F32, tag="deg")
                    nc.vector.tensor_reduce(out=deg[:], in_=thr[:, :C],
                                            axis=mybir.AxisListType.X,
                                            op=OP.add)
                    # guarded 1/rs
                    g01 = wpool.tile([P, 1], F32, tag="g01")
                    nc.vector.tensor_scalar(out=g01[:], in0=rs[:], scalar1=0.0,
                                            scalar2=None, op0=OP.is_gt)
                    nc.vector.scalar_tensor_tensor(
                        out=rs[:], in0=rs[:], scalar=1.0, in1=g01[:],
                        op0=OP.subtract, op1=OP.mult)
                    nc.vector.tensor_scalar_add(rs[:], rs[:], 1.0)
                    nc.vector.reciprocal(rs[:], rs[:])
                    att = wpool.tile([P, CWmax], F32, tag="att")
                    nc.vector.tensor_scalar(out=att[:, :C], in0=sims[:, :C],
                                            scalar1=rs[:], scalar2=None,
                                            op0=OP.mult)
                    we = wpool.tile([P, CWmax], F32, tag="we")
                    nc.scalar.activation(out=we[:, :C], in_=att[:, :C],
                                         func=AF.Exp)
                    nc.vector.tensor_tensor(out=we[:, :C], in0=we[:, :C],
                                            in1=thr[:, :C], op=OP.mult)
                    wL = wpool.tile([P, CWmax], F32, tag="wL")
                    nc.vector.tensor_tensor(out=wL[:, :C], in0=we[:, :C],
                                            in1=mL_sb[:, ow:ow + C],
                                            op=OP.mult)
                    wR = wpool.tile([P, CWmax], F32, tag="wR")
                    nc.vector.tensor_tensor(out=wR[:, :C], in0=we[:, :C],
                                            in1=wL[:, :C], op=OP.subtract)
                    # lam / w_diag
                    wd = wpool.tile([P, 1], F32, tag="wd")
                    nc.vector.tensor_scalar_add(wd[:], deg[:], 1.0)
                    nc.vector.reciprocal(wd[:], wd[:])
                    nc.scalar.activation(out=wd[:], in_=wd[:], func=AF.Exp)
                    # agg^T via PE: sum_c s_half^T(lhsT) x diag(w)
                    aggT_ps = psAG.tile([P, P], F32, tag="aggT")
                    for c in range(C):
                        dL = dpool.tile([P, P], F16, tag="dL")
                        nc.vector.tensor_scalar(
                            out=dL[:], in0=ident16[:],
                            scalar1=wL[:, c:c + 1], scalar2=None, op0=OP.mult)
                        nc.tensor.matmul(out=aggT_ps[:],
                                         lhsT=sC[:, c, D:2 * D], rhs=dL[:],
                                         start=(c == 0), stop=False)
                        dR = dpool.tile([P, P], F16, tag="dR")
                        nc.scalar.activation(out=dR[:], in_=ident16[:],
                                             func=AF.Identity,
                                             scale=wR[:, c:c + 1])
                        nc.tensor.matmul(out=aggT_ps[:],
                                         lhsT=sC[:, c, 3 * D:4 * D], rhs=dR[:],
                                         start=False, stop=(c == C - 1))
                    aggT_sb = spool.tile([P, D], F32, tag="aggT_sb")
                    nc.scalar.copy(aggT_sb[:], aggT_ps[:])
                    agg_ps = psA2.tile([P, P], F32, tag="agg")
                    nc.tensor.transpose(out=agg_ps[:], in_=aggT_sb[:],
                                        identity=ident[:])
                    h2 = hpool.tile([P, D], F32, tag="h2")
                    nc.vector.scalar_tensor_tensor(
                        out=h2[:], in0=sloc, scalar=wd[:], in1=agg_ps[:],
                        op0=OP.mult, op1=OP.add)
                if not b_zero:
                    nc.vector.tensor_tensor(
                        out=h2[:], in0=h2[:],
                        in1=b_sb[0 if layer == 0 else 1][:], op=OP.add)
                if layer < 2:
                    st6 = wpool.tile([P, 6], F32, tag="st6")
                    nc.vector.bn_stats(out=st6[:], in_=h2[:])
                    mv = wpool.tile([P, 2], F32, tag="mv")
                    nc.vector.bn_aggr(out=mv[:], in_=st6[:])
                    sd = wpool.tile([P, 1], F32, tag="sd")
                    nc.scalar.activation(out=sd[:], in_=mv[:, 1:2],
                                         func=AF.Sqrt, bias=constcol(EPS))
                    nc.vector.reciprocal(sd[:], sd[:])
                    if ln_trivial:
                        # relu((h - mu) * isd) in one Act op
                        nb = wpool.tile([P, 1], F32, tag="nb")
                        nc.vector.tensor_scalar(
                            out=nb[:], in0=mv[:, 0:1], scalar1=sd[:],
                            scalar2=-1.0, op0=OP.mult, op1=OP.mult)
                        nc.scalar.activation(out=h2[:], in_=h2[:],
                                             func=AF.Relu, bias=nb[:],
                                             scale=sd[:])
                    else:
                        nc.vector.tensor_scalar(
                            out=h2[:], in0=h2[:], scalar1=mv[:, 0:1],
                            scalar2=sd[:], op0=OP.subtract, op1=OP.mult)
                        nc.vector.tensor_tensor(out=h2[:], in0=h2[:],
                                                in1=lng_sb[layer][:],
                                                op=OP.mult)
                        nc.vector.tensor_tensor(out=h2[:], in0=h2[:],
                                                in1=lnb_sb[layer][:],
                                                op=OP.add)
                        nc.vector.tensor_scalar(out=h2[:], in0=h2[:],
                                                scalar1=0.0, scalar2=None,
                                                op0=OP.max)
                    node_emit(h2, w, layer + 1)
                    emit_chunk_cc(layer + 1, w)
                else:
                    mx = wpool.tile([P, 1], F32, tag="mx")
                    nc.vector.tensor_reduce(out=mx[:], in_=h2[:],
                                            axis=mybir.AxisListType.X,
                                            op=OP.max)
                    nc.vector.tensor_scalar_mul(mx[:], mx[:], -1.0)
                    ex = spool.tile([P, D], F32, tag="ex")
                    se = wpool.tile([P, 1], F32, tag="se")
                    nc.scalar.activation(out=ex[:], in_=h2[:], func=AF.Exp,
                                         bias=mx[:], accum_out=se[:])
                    nc.scalar.activation(out=se[:], in_=se[:], func=AF.Ln)
                    nc.vector.tensor_tensor(out=mx[:], in0=mx[:], in1=se[:],
                                            op=OP.subtract)
                    nc.vector.tensor_scalar_add(h2[:], h2[:], mx[:])
                    nc.sync.dma_start(out_t[w * P:(w + 1) * P, :], h2[:])

    nc.compile()
    return nc


# ---------------------------------------------------------------------------
# fast-path entry helpers
# ---------------------------------------------------------------------------

_CACHE_FAST = {}


def _kernel_fast(inputs, prep, ln_trivial, b_zero):
    from concourse.bass_utils import run_bass_kernel_spmd

    R, W = prep["R"], prep["W"]
    gsplit = int(os.environ.get("GG_GSPLIT", "0")) or 6
    key = (R, W, prep["CW"], prep["CHB"], ln_trivial, b_zero, gsplit)
    if key not in _CACHE_FAST:
        _CACHE_FAST[key] = _build_fast(R, W, prep["CW"], prep["CHB"],
                                       ln_trivial, b_zero, gsplit)
    nc = _CACHE_FAST[key]

    x = np.ascontiguousarray(np.asarray(inputs["x"], dtype=np.float32))
    n = x.shape[0]
    lng = np.stack([np.asarray(inputs["ln1_g"], np.float32),
                    np.asarray(inputs["ln2_g"], np.float32)])
    lnb = np.stack([np.asarray(inputs["ln1_b"], np.float32),
                    np.asarray(inputs["ln2_b"], np.float32)])
    in_maps = []
    for c in range(NC):
        order = prep["orders"][c]
        xp = np.zeros((R, D), np.float32)
        xp[:order.shape[0]] = x[order]
        in_maps.append({
            "x": xp,
            "W0": np.ascontiguousarray(np.asarray(inputs["W0"], np.float32)),
            "W1": np.ascontiguousarray(np.asarray(inputs["W1"], np.float32)),
            "b0": np.asarray(inputs["b0"], np.float32).reshape(1, D).copy(),
            "b1": np.asarray(inputs["b1"], np.float32).reshape(1, D).copy(),
            "idx16": prep["idx16"][c],
            "mL": prep["mL"][c], "mR": prep["mR"][c],
            "lng": np.ascontiguousarray(lng), "lnb": np.ascontiguousarray(lnb),
        })
    res = run_bass_kernel_spmd(nc, in_maps, core_ids=list(range(NC)),
                               trace=bool(int(os.environ.get("GG_TRACE", "0"))))
    out = np.empty((n, D), np.float32)
    for c in range(NC):
        order = prep["orders"][c]
        out[order] = res.results[c]["out"][:order.shape[0]]
    if os.environ.get("GG_RESULT_OBJ"):
        kernel._last_results = res
    return out


# ===========================================================================
# general fallback path (original implementation, unchanged)
# ===========================================================================

SW = 192          # s-table row width (s[128] | rs | pad)


def _preprocess(row, col, n_nodes):
    row = np.asarray(row).astype(np.int64)
    col = np.asarray(col).astype(np.int64)
    E = row.shape[0]
    R = int(np.ceil(n_nodes / NC / P)) * P
    W = R // P
    NPAD = NC * R

    keys = np.sort(row * n_nodes + col)
    rkeys = col * n_nodes + row
    pos = np.clip(np.searchsorted(keys, rkeys), 0, E - 1)
    has_rev_e = (keys[pos] == rkeys).astype(np.float32)

    order = np.lexsort((col, row))
    srow, scol, shrev = row[order], col[order], has_rev_e[order]

    chunk = srow // R
    lr = srow - chunk * R
    win = lr // P
    rel = lr % P
    gw = chunk * W + win
    cnt = np.bincount(gw, minlength=NC * W)
    K = max(1, int(np.ceil(cnt.max() / P)))
    S = K * P

    starts = np.zeros(NC * W, dtype=np.int64)
    starts[1:] = np.cumsum(cnt)[:-1]
    slot = gw * S + (np.arange(E) - starts[gw])

    colid = np.zeros(NC * W * S, np.int64)       # pads -> row 0 (+ vmask)
    relc = np.full(NC * W * S, P - 1, np.float32)
    hrev = np.zeros(NC * W * S, np.float32)
    vmask = np.zeros(NC * W * S, np.float32)
    mleft = np.ones(NC * W * S, np.float32)

    colid[slot] = scol // 2                       # pair-packed row id
    mleft[slot] = (scol % 2 == 0).astype(np.float32)
    relc[slot] = rel
    hrev[slot] = shrev
    vmask[slot] = 1.0

    def per_core_pk(arr):
        a = arr.reshape(NC, W, K, P)
        return [np.ascontiguousarray(a[c].transpose(2, 0, 1).reshape(P, W * K))
                for c in range(NC)]

    idx16 = [np.concatenate(
        [_pack_idx16(colid[(c * W + w) * S:(c * W + w + 1) * S])
         for w in range(W)], axis=1) for c in range(NC)]

    return dict(
        R=R, W=W, K=K, S=S, NPAD=NPAD, E=E,
        idx16=idx16, relc=per_core_pk(relc), hrev=per_core_pk(hrev),
        vmask=per_core_pk(vmask), mleft=per_core_pk(mleft),
    )


def _build(R, W, K, wd0, wd1, bd, ln_trivial, b_zero):
    import concourse.bass as bass  # noqa: F401
    import concourse.bacc as bacc
    import concourse.mybir as mybir
    import concourse.tile as tile
    from concourse.masks import make_identity

    F32 = mybir.dt.float32
    I16 = mybir.dt.int16
    AF = mybir.ActivationFunctionType
    OP = mybir.AluOpType

    S = K * P
    NPAD = NC * R
    NRS = NC * R
    RG = [list(range(NC))]
    SC = S // 16                     # idx16 columns per window

    nc = bacc.Bacc("TRN2", target_bir_lowering=False)

    x_in = nc.dram_tensor("x", [R, D], F32, kind="ExternalInput")
    w0_in = nc.dram_tensor("W0", [D, D], F32, kind="ExternalInput")
    w1_in = nc.dram_tensor("W1", [D, D], F32, kind="ExternalInput")
    b0_in = nc.dram_tensor("b0", [1, D], F32, kind="ExternalInput")
    b1_in = nc.dram_tensor("b1", [1, D], F32, kind="ExternalInput")
    idx_in = nc.dram_tensor("idx16", [P, W * SC], I16, kind="ExternalInput")
    relc_in = nc.dram_tensor("relc", [P, W * K], F32, kind="ExternalInput")
    hrev_in = nc.dram_tensor("hrev", [P, W * K], F32, kind="ExternalInput")
    vmask_in = nc.dram_tensor("vmask", [P, W * K], F32, kind="ExternalInput")
    mleft_in = nc.dram_tensor("mleft", [P, W * K], F32, kind="ExternalInput")
    lng_in = nc.dram_tensor("lng", [2, D], F32, kind="ExternalInput")
    lnb_in = nc.dram_tensor("lnb", [2, D], F32, kind="ExternalInput")
    out_t = nc.dram_tensor("out", [R, D], F32, kind="ExternalOutput")

    TABH = nc.dram_tensor("tabh", [NPAD, D], F32, kind="Internal",
                          addr_space="Shared")
    TABS = nc.dram_tensor("tabs", [NPAD, SW], F32, kind="Internal",
                          addr_space="Shared")
    rs_tab = nc.dram_tensor("rstab", [NRS, 1], F32, kind="Internal",
                            addr_space="Shared")
    con_h = [nc.dram_tensor(f"conh{i}", [R, D], F32, kind="Internal")
             for i in range(2)]
    con_s = [nc.dram_tensor(f"cons{i}", [R, SW], F32, kind="Internal")
             for i in range(2)]
    rs_con = nc.dram_tensor("rscon", [W, P], F32, kind="Internal")
    rden_d = nc.dram_tensor("rdend", [W, P], F32, kind="Internal")

    with tile.TileContext(nc) as tc, ExitStack() as ctx:
        singles = ctx.enter_context(tc.tile_pool(name="singles", bufs=1))
        hpool = ctx.enter_context(tc.tile_pool(name="hpool", bufs=3))
        gpool = ctx.enter_context(tc.tile_pool(name="gpool", bufs=4))
        ipool = ctx.enter_context(tc.tile_pool(name="ipool", bufs=2))
        spool = ctx.enter_context(tc.tile_pool(name="spool", bufs=3))
        wpool = ctx.enter_context(tc.tile_pool(name="wpool", bufs=4))
        psTR = ctx.enter_context(tc.tile_pool(name="psTR", bufs=1, space="PSUM"))
        psIT = ctx.enter_context(tc.tile_pool(name="psIT", bufs=2, space="PSUM"))
        psHR = ctx.enter_context(tc.tile_pool(name="psHR", bufs=2, space="PSUM"))
        psAG = ctx.enter_context(tc.tile_pool(name="psAG", bufs=1, space="PSUM"))
        psSM = ctx.enter_context(tc.tile_pool(name="psSM", bufs=1, space="PSUM"))

        ident = singles.tile([P, P], F32)
        make_identity(nc, ident[:])
        iota = singles.tile([P, P], mybir.dt.int32)
        nc.gpsimd.iota(iota[:], pattern=[[1, P]], base=0, channel_multiplier=0)
        iota_f = singles.tile([P, P], F32)
        nc.vector.tensor_copy(iota_f[:], iota[:])

        _consts = {}

        def constcol(val):
            if val not in _consts:
                t = singles.tile([P, 1], F32, tag=f"const{len(_consts)}")
                nc.vector.memset(t[:], float(val))
                _consts[val] = t
            return _consts[val][:]

        w0_sb = singles.tile([D, D], F32)
        nc.sync.dma_start(w0_sb[:], w0_in[:, :])
        w1_sb = singles.tile([D, D], F32)
        nc.sync.dma_start(w1_sb[:], w1_in[:, :])
        b_sb = []
        for t_in in (b0_in, b1_in):
            t = singles.tile([P, D], F32)
            nc.gpsimd.dma_start(t[:], t_in[0:1, :].to_broadcast([P, D]))
            b_sb.append(t)
        lng_sb = [None, None]
        lnb_sb = [None, None]
        if not ln_trivial:
            for i in range(2):
                g = singles.tile([P, D], F32, tag=f"lng{i}")
                nc.gpsimd.dma_start(g[:], lng_in[i:i + 1, :].to_broadcast([P, D]))
                lng_sb[i] = g
                b = singles.tile([P, D], F32, tag=f"lnb{i}")
                nc.gpsimd.dma_start(b[:], lnb_in[i:i + 1, :].to_broadcast([P, D]))
                lnb_sb[i] = b

        idx_sb = singles.tile([P, W * SC], I16)
        nc.sync.dma_start(idx_sb[:], idx_in[:, :])
        relc_sb = singles.tile([P, W * K], F32)
        nc.sync.dma_start(relc_sb[:], relc_in[:, :])
        hrev_sb = singles.tile([P, W * K], F32)
        nc.sync.dma_start(hrev_sb[:], hrev_in[:, :])
        vmask_sb = singles.tile([P, W * K], F32)
        nc.sync.dma_start(vmask_sb[:], vmask_in[:, :])
        mleft_sb = singles.tile([P, W * K], F32)
        nc.sync.dma_start(mleft_sb[:], mleft_in[:, :])

        sims = singles.tile([P, W * K], F32)

        zpad = singles.tile([P, SW - D], F32)
        nc.vector.memset(zpad[:], 0.0)
        for ci in range(2):
            for w in range(W):
                nc.sync.dma_start(con_s[ci][w * P:(w + 1) * P, D:], zpad[:])

        def node_ops(h_sb, w, layer_next):
            dsth = con_h[layer_next % 2]
            dsts = con_s[layer_next % 2]
            wmat = w0_sb if layer_next == 0 else w1_sb
            ss = wpool.tile([P, 1], F32, tag="ss")
            scr = spool.tile([P, D], F32, tag="nscr")
            nc.vector.scalar_tensor_tensor(
                out=scr[:], in0=h_sb[:], scalar=1.0, in1=h_sb[:],
                op0=OP.mult, op1=OP.mult, accum_out=ss[:])
            nc.scalar.activation(out=ss[:], in_=ss[:], func=AF.Sqrt,
                                 bias=constcol(1e-30))
            nc.vector.reciprocal(ss[:], ss[:])
            hn = spool.tile([P, D], F32, tag="hn")
            nc.vector.tensor_scalar_mul(hn[:], h_sb[:], ss[:])
            nc.sync.dma_start(dsth[w * P:(w + 1) * P, :], hn[:])
            hT_ps = psTR.tile([P, P], F32, tag="tr")
            nc.tensor.transpose(out=hT_ps[:], in_=h_sb[:], identity=ident[:])
            hT = spool.tile([P, D], F32, tag="hT")
            nc.scalar.copy(hT[:], hT_ps[:])
            s_ps = psTR.tile([P, P], F32, tag="tr")
            nc.tensor.matmul(out=s_ps[:], lhsT=hT[:], rhs=wmat[:],
                             start=True, stop=True)
            s_sb = spool.tile([P, D], F32, tag="s_sb")
            nc.scalar.copy(s_sb[:], s_ps[:])
            nc.sync.dma_start(dsts[w * P:(w + 1) * P, :D], s_sb[:])

        for w in range(W):
            h_sb = hpool.tile([P, D], F32, tag="h0")
            nc.sync.dma_start(h_sb[:], x_in[w * P:(w + 1) * P, :])
            node_ops(h_sb, w, 0)

        for layer in range(3):
            ch = con_h[layer % 2]
            cs = con_s[layer % 2]
            bias = b_sb[0] if layer == 0 else b_sb[1]

            nc.gpsimd.collective_compute(
                "AllGather", OP.bypass, replica_groups=RG,
                ins=[ch[:, :]], outs=[TABH[:NPAD, :]])
            nc.gpsimd.collective_compute(
                "AllGather", OP.bypass, replica_groups=RG,
                ins=[cs[:, :]], outs=[TABS[:NPAD, :]])

            # ---------- B1: sims + rs ----------
            for w in range(W):
                hnC = gpool.tile([P, K, 2 * D], F32, tag="hnC")
                for t0 in range(0, K, 6):
                    t1 = min(t0 + 6, K)
                    nc.gpsimd.dma_gather(
                        out_ap=hnC[:, t0:t1, :],
                        in_ap=TABH[:, :].rearrange("(a b) d -> a (b d)", b=2),
                        idxs_ap=idx_sb[:, w * SC + t0 * 8:w * SC + t1 * 8],
                        num_idxs=(t1 - t0) * P, num_idxs_reg=(t1 - t0) * P,
                        elem_size=2 * D)
                hnW = wpool.tile([P, D], F32, tag="hnW")
                nc.sync.dma_start(hnW[:], ch[w * P:(w + 1) * P, :])
                I_w = ipool.tile([P, S], F32, tag="I_w")
                simL = wpool.tile([P, K], F32, tag="simL")
                simR = wpool.tile([P, K], F32, tag="simR")
                for t in range(K):
                    c0 = w * K + t
                    nc.vector.tensor_scalar(
                        out=I_w[:, t * P:(t + 1) * P], in0=iota_f[:],
                        scalar1=relc_sb[:, c0:c0 + 1], scalar2=None,
                        op0=OP.is_equal)
                    IT_ps = psIT.tile([P, P], F32, tag="IT")
                    nc.tensor.transpose(out=IT_ps[:],
                                        in_=I_w[:, t * P:(t + 1) * P],
                                        identity=ident[:])
                    IT = wpool.tile([P, P], F32, tag="ITsb")
                    nc.scalar.copy(IT[:], IT_ps[:])
                    hre_ps = psHR.tile([P, P], F32, tag="hre")
                    nc.tensor.matmul(out=hre_ps[:], lhsT=IT[:], rhs=hnW[:],
                                     start=True, stop=True)
                    scr = spool.tile([P, D], F32, tag="simscr")
                    nc.vector.scalar_tensor_tensor(
                        out=scr[:], in0=hnC[:, t, :D], scalar=1.0,
                        in1=hre_ps[:], op0=OP.mult, op1=OP.mult,
                        accum_out=simL[:, t:t + 1])
                    nc.vector.scalar_tensor_tensor(
                        out=scr[:], in0=hnC[:, t, D:], scalar=1.0,
                        in1=hre_ps[:], op0=OP.mult, op1=OP.mult,
                        accum_out=simR[:, t:t + 1])
                cw = slice(w * K, (w + 1) * K)
                nc.vector.tensor_tensor(out=simL[:], in0=simL[:], in1=simR[:],
                                        op=OP.subtract)
                nc.vector.tensor_tensor(out=simL[:], in0=simL[:],
                                        in1=mleft_sb[:, cw], op=OP.mult)
                nc.vector.tensor_tensor(out=sims[:, cw], in0=simL[:],
                                        in1=simR[:], op=OP.add)
                thr = wpool.tile([P, K], F32, tag="thr")
                nc.vector.tensor_scalar(out=thr[:], in0=sims[:, cw],
                                        scalar1=0.1, scalar2=None, op0=OP.is_ge)
                nc.vector.tensor_tensor(out=thr[:], in0=thr[:],
                                        in1=vmask_sb[:, cw], op=OP.mult)
                nc.vector.tensor_tensor(out=sims[:, cw], in0=sims[:, cw],
                                        in1=thr[:], op=OP.mult)
                rs_ps = psSM.tile([1, P], F32, tag="rs")
                for t in range(K):
                    c0 = w * K + t
                    nc.tensor.matmul(out=rs_ps[:], lhsT=sims[:, c0:c0 + 1],
                                     rhs=I_w[:, t * P:(t + 1) * P],
                                     start=(t == 0), stop=(t == K - 1))
                rs_sb = wpool.tile([1, P], F32, tag="rs_sb")
                nc.scalar.copy(rs_sb[:], rs_ps[:])
                nc.sync.dma_start(rs_con[w:w + 1, :], rs_sb[:])

            nc.gpsimd.collective_compute(
                "AllGather", OP.bypass, replica_groups=RG,
                ins=[rs_con[:, :]], outs=[rs_tab[:NRS, :]])
            with nc.allow_non_contiguous_dma(reason="rs column scatter"):
                for ci in range(NC):
                    nc.sync.dma_start(
                        TABS[ci * R:(ci + 1) * R, D:D + 1],
                        rs_tab[ci * R:(ci + 1) * R, :])

            # ---------- B2: att, mask, conv ----------
            for w in range(W):
                cw = slice(w * K, (w + 1) * K)
                sC = gpool.tile([P, K, 2 * SW], F32, tag="sC")
                for t0 in range(0, K, 6):
                    t1 = min(t0 + 6, K)
                    nc.gpsimd.dma_gather(
                        out_ap=sC[:, t0:t1, :],
                        in_ap=TABS[:, :].rearrange("(a b) d -> a (b d)", b=2),
                        idxs_ap=idx_sb[:, w * SC + t0 * 8:w * SC + t1 * 8],
                        num_idxs=(t1 - t0) * P, num_idxs_reg=(t1 - t0) * P,
                        elem_size=2 * SW)
                rsr = wpool.tile([1, P], F32, tag="rsrow")
                nc.sync.dma_start(rsr[:], rs_con[w:w + 1, :])
                g01 = wpool.tile([1, P], F32, tag="g01")
                nc.vector.tensor_scalar(out=g01[:], in0=rsr[:], scalar1=0.0,
                                        scalar2=None, op0=OP.is_gt)
                nc.vector.scalar_tensor_tensor(
                    out=rsr[:], in0=rsr[:], scalar=1.0, in1=g01[:],
                    op0=OP.subtract, op1=OP.mult)
                nc.vector.tensor_scalar_add(rsr[:], rsr[:], 1.0)
                nc.vector.reciprocal(rsr[:], rsr[:])
                nc.sync.dma_start(rden_d[w:w + 1, :], rsr[:])
                rden_col = wpool.tile([P, 1], F32, tag="rdenc")
                nc.sync.dma_start(rden_col[:, :], rden_d[w, :, None])

                att = wpool.tile([P, K], F32, tag="att")
                rev = wpool.tile([P, K], F32, tag="rev")
                scr = wpool.tile([P, K], F32, tag="mscr")
                rde = wpool.tile([P, K], F32, tag="rde")
                for t in range(K):
                    c0 = w * K + t
                    I_t = ipool.tile([P, P], F32, tag="I_t")
                    nc.vector.tensor_scalar(
                        out=I_t[:], in0=iota_f[:],
                        scalar1=relc_sb[:, c0:c0 + 1], scalar2=None,
                        op0=OP.is_equal)
                    IT_ps = psIT.tile([P, P], F32, tag="IT")
                    nc.tensor.transpose(out=IT_ps[:], in_=I_t[:],
                                        identity=ident[:])
                    IT = wpool.tile([P, P], F32, tag="ITsb")
                    nc.scalar.copy(IT[:], IT_ps[:])
                    rex_ps = psHR.tile([P, P], F32, tag="hre")
                    nc.tensor.matmul(out=rex_ps[:, 0:1], lhsT=IT[:],
                                     rhs=rden_col[:], start=True, stop=True)
                    nc.scalar.copy(rde[:, t:t + 1], rex_ps[:, 0:1])
                nc.vector.tensor_tensor(out=att[:], in0=sims[:, cw],
                                        in1=rde[:], op=OP.mult)
                rs_c = wpool.tile([P, K], F32, tag="rs_c")
                nc.vector.tensor_tensor(out=rs_c[:], in0=sC[:, :, D],
                                        in1=sC[:, :, SW + D], op=OP.subtract)
                nc.vector.tensor_tensor(out=rs_c[:], in0=rs_c[:],
                                        in1=mleft_sb[:, cw], op=OP.mult)
                nc.vector.tensor_tensor(out=rs_c[:], in0=rs_c[:],
                                        in1=sC[:, :, SW + D], op=OP.add)
                nc.vector.tensor_scalar(out=scr[:], in0=rs_c[:], scalar1=0.0,
                                        scalar2=None, op0=OP.is_gt)
                nc.vector.scalar_tensor_tensor(
                    out=rev[:], in0=rs_c[:], scalar=1.0, in1=scr[:],
                    op0=OP.subtract, op1=OP.mult)
                nc.vector.tensor_scalar_add(rev[:], rev[:], 1.0)
                nc.vector.reciprocal(rev[:], rev[:])
                nc.vector.tensor_tensor(out=rev[:], in0=rev[:],
                                        in1=sims[:, cw], op=OP.mult)
                nc.vector.tensor_tensor(out=rev[:], in0=rev[:],
                                        in1=hrev_sb[:, cw], op=OP.mult)
                nc.scalar.activation(out=rev[:], in_=rev[:], func=AF.Identity,
                                     bias=constcol(bd), scale=wd1)
                nc.vector.scalar_tensor_tensor(
                    out=scr[:], in0=att[:], scalar=wd0, in1=rev[:],
                    op0=OP.mult, op1=OP.add)
                nc.vector.tensor_scalar(out=scr[:], in0=scr[:], scalar1=0.0,
                                        scalar2=None, op0=OP.is_gt)
                nc.vector.tensor_tensor(out=att[:], in0=att[:], in1=scr[:],
                                        op=OP.mult)
                nc.vector.tensor_scalar(out=scr[:], in0=att[:], scalar1=0.0,
                                        scalar2=None, op0=OP.not_equal)
                nc.scalar.activation(out=att[:], in_=att[:], func=AF.Exp)
                nc.vector.tensor_tensor(out=att[:], in0=att[:], in1=scr[:],
                                        op=OP.mult)
                attL = wpool.tile([P, K], F32, tag="attL")
                attR = wpool.tile([P, K], F32, tag="attR")
                nc.vector.tensor_tensor(out=attL[:], in0=att[:],
                                        in1=mleft_sb[:, cw], op=OP.mult)
                nc.vector.tensor_tensor(out=attR[:], in0=att[:],
                                        in1=attL[:], op=OP.subtract)
                agg_ps = psAG.tile([P, P + 1], F32, tag="agg")
                for t in range(K):
                    c0 = w * K + t
                    I_t = ipool.tile([P, P], F32, tag="I_t2")
                    nc.vector.tensor_scalar(
                        out=I_t[:], in0=iota_f[:],
                        scalar1=relc_sb[:, c0:c0 + 1], scalar2=None,
                        op0=OP.is_equal)
                    wsc = spool.tile([P, P + 1], F32, tag="wsc")
                    nc.vector.tensor_scalar_mul(
                        wsc[:, :D], sC[:, t, :D], attL[:, t:t + 1])
                    nc.vector.scalar_tensor_tensor(
                        out=wsc[:, :D], in0=sC[:, t, SW:SW + D],
                        scalar=attR[:, t:t + 1], in1=wsc[:, :D],
                        op0=OP.mult, op1=OP.add)
                    nc.vector.tensor_copy(wsc[:, D:D + 1], scr[:, t:t + 1])
                    nc.tensor.matmul(out=agg_ps[:], lhsT=I_t[:], rhs=wsc[:],
                                     start=(t == 0), stop=(t == K - 1))
                lam = wpool.tile([P, 1], F32, tag="lam")
                nc.vector.tensor_scalar_add(lam[:], agg_ps[:, D:D + 1], 1.0)
                nc.vector.reciprocal(lam[:], lam[:])
                nc.scalar.activation(out=lam[:], in_=lam[:], func=AF.Exp)
                s_loc = spool.tile([P, D], F32, tag="s_loc")
                nc.sync.dma_start(s_loc[:], cs[w * P:(w + 1) * P, :D])
                h2 = hpool.tile([P, D], F32, tag="h2")
                nc.vector.scalar_tensor_tensor(
                    out=h2[:], in0=s_loc[:], scalar=lam[:], in1=agg_ps[:, :D],
                    op0=OP.mult, op1=OP.add)
                if not b_zero:
                    nc.vector.tensor_tensor(out=h2[:], in0=h2[:], in1=bias[:],
                                            op=OP.add)
                if layer < 2:
                    st6 = wpool.tile([P, 6], F32, tag="st6")
                    nc.vector.bn_stats(out=st6[:], in_=h2[:])
                    mv = wpool.tile([P, 2], F32, tag="mv")
                    nc.vector.bn_aggr(out=mv[:], in_=st6[:])
                    sd = wpool.tile([P, 1], F32, tag="sd")
                    nc.scalar.activation(out=sd[:], in_=mv[:, 1:2],
                                         func=AF.Sqrt, bias=constcol(EPS))
                    nc.vector.reciprocal(sd[:], sd[:])
                    nc.vector.tensor_scalar(
                        out=h2[:], in0=h2[:], scalar1=mv[:, 0:1],
                        scalar2=sd[:], op0=OP.subtract, op1=OP.mult)
                    if not ln_trivial:
                        nc.vector.tensor_tensor(out=h2[:], in0=h2[:],
                                                in1=lng_sb[layer][:],
                                                op=OP.mult)
                        nc.vector.tensor_tensor(out=h2[:], in0=h2[:],
                                                in1=lnb_sb[layer][:],
                                                op=OP.add)
                    nc.scalar.activation(out=h2[:], in_=h2[:], func=AF.Relu)
                    node_ops(h2, w, layer + 1)
                else:
                    mx = wpool.tile([P, 1], F32, tag="mx")
                    nc.vector.tensor_reduce(out=mx[:], in_=h2[:],
                                            axis=mybir.AxisListType.X,
                                            op=OP.max)
                    nc.vector.tensor_scalar_mul(mx[:], mx[:], -1.0)
                    ex = spool.tile([P, D], F32, tag="ex")
                    se = wpool.tile([P, 1], F32, tag="se")
                    nc.scalar.activation(out=ex[:], in_=h2[:], func=AF.Exp,
                                         bias=mx[:], accum_out=se[:])
                    nc.scalar.activation(out=se[:], in_=se[:], func=AF.Ln)
                    nc.vector.tensor_tensor(out=mx[:], in0=mx[:], in1=se[:],
                                            op=OP.subtract)
                    nc.vector.tensor_scalar_add(h2[:], h2[:], mx[:])
                    nc.sync.dma_start(out_t[w * P:(w + 1) * P, :], h2[:])

    nc.compile()
    return nc


_CACHE = {}


def _get_built(key, R, W, K, wd0, wd1, bd, ln_trivial, b_zero):
    if key not in _CACHE:
        _CACHE[key] = _build(R, W, K, wd0, wd1, bd, ln_trivial, b_zero)
    return _CACHE[key]


def make_in_maps(inputs, prep):
    x = np.ascontiguousarray(np.asarray(inputs["x"], dtype=np.float32))
    n = x.shape[0]
    R = prep["R"]
    xp = np.zeros((NC * R, D), np.float32)
    xp[:n] = x
    lng = np.stack([np.asarray(inputs["ln1_g"], np.float32),
                    np.asarray(inputs["ln2_g"], np.float32)])
    lnb = np.stack([np.asarray(inputs["ln1_b"], np.float32),
                    np.asarray(inputs["ln2_b"], np.float32)])
    in_maps = []
    for c in range(NC):
        in_maps.append({
            "x": np.ascontiguousarray(xp[c * R:(c + 1) * R]),
            "W0": np.ascontiguousarray(np.asarray(inputs["W0"], np.float32)),
            "W1": np.ascontiguousarray(np.asarray(inputs["W1"], np.float32)),
            "b0": np.asarray(inputs["b0"], np.float32).reshape(1, D).copy(),
            "b1": np.asarray(inputs["b1"], np.float32).reshape(1, D).copy(),
            "idx16": prep["idx16"][c],
            "relc": prep["relc"][c], "hrev": prep["hrev"][c],
            "vmask": prep["vmask"][c], "mleft": prep["mleft"][c],
            "lng": np.ascontiguousarray(lng), "lnb": np.ascontiguousarray(lnb),
        })
    return in_maps


def _get_params(inputs):
    wd0 = float(np.asarray(inputs["drop_W"])[0, 0])
    wd1 = float(np.asarray(inputs["drop_W"])[0, 1])
    bd = float(np.asarray(inputs["drop_b"]).reshape(-1)[0])
    ln_trivial = all(
        np.all(np.asarray(inputs[k]) == v)
        for k, v in (("ln1_g", 1), ("ln2_g", 1), ("ln1_b", 0), ("ln2_b", 0)))
    b_zero = (np.all(np.asarray(inputs["b0"]) == 0)
              and np.all(np.asarray(inputs["b1"]) == 0))
    return wd0, wd1, bd, ln_trivial, b_zero


def kernel(**inputs):
    from concourse.bass_utils import run_bass_kernel_spmd

    row = np.asarray(inputs["row"])
    col = np.asarray(inputs["col"])
    n = np.asarray(inputs["x"]).shape[0]
    wd0, wd1, bd, ln_trivial, b_zero = _get_params(inputs)

    # drop gate is a no-op iff z = att*wd0 + att_rev*wd1 + bd > 0 whenever
    # att > 0 (given att, att_rev >= 0): wd0 > 0, wd1 >= 0, bd >= 0.
    if wd0 > 0 and wd1 >= 0 and bd >= 0 and n % NC == 0:
        prep = _prep_fast(row, col, n)
        return _kernel_fast(inputs, prep, ln_trivial, b_zero).astype(np.float32)

    prep = _preprocess(row, col, n)
    key = (n, prep["R"], prep["K"], wd0, wd1, bd, ln_trivial, b_zero)
    nc = _get_built(key, prep["R"], prep["W"], prep["K"], wd0, wd1, bd,
                    ln_trivial, b_zero)
    in_maps = make_in_maps(inputs, prep)
    res = run_bass_kernel_spmd(nc, in_maps, core_ids=list(range(NC)),
                               trace=bool(int(os.environ.get("GG_TRACE", "0"))))
    out = np.concatenate([r["out"] for r in res.results], axis=0)[:n]
    if os.environ.get("GG_RESULT_OBJ"):
        kernel._last_results = res
    return out.astype(np.float32)


# revision 17
# speedup vs baseline: 1.0320x; 1.0320x over previous
"""GCNGuard forward on 8 Trainium2 NeuronCores (Bass/Tile) — fast path.

Key observation: with drop_W = [w0 > 0, w1 >= 0] and drop_b = 0 the learned
drop gate is a no-op (z = att*w0 + att_rev*w1 >= 0, and z > 0 whenever
att > 0, so att * (sigmoid(z) > .5) == att).  The whole reverse-edge /
rs-AllGather machinery of the general path is dead: each layer is a single
fused pass needing only row-local quantities plus gathered [hn | s] rows of
the edge's col endpoint.

Layout: nodes sharded 6250/core, degree-sorted into 49 windows of 128 rows.
Edges live at (window, column j, partition r) = row-major slots, so the
one-hot scatter matrix is the identity: rs/deg are free-axis reduces, and
agg^T accumulates on the PE as sum_c matmul(lhsT=s_gathered[c], rhs=
diag(w[:, c])).  Per layer each core publishes its [hn16 | s16] rows into a
shared table via NCHUNK AllGathers (chunk-major layout keeps each collective
contiguous) overlapped behind the edge pass; edges gather 1 KiB fp16 pairs
(pair id fits int16).  Sims: per-column stt with the pair-select mask folded
into the per-partition scalar (simL on DVE, simR on GpSimd); diag builds
split DVE/Act.  fp16 tables measured at 5.9e-4 end-to-end rel err.
"""

import os
from contextlib import ExitStack

import numpy as np

P = 128
D = 128
NC = 8
EPS = 1e-5

# ---------------------------------------------------------------------------
# host-side preprocessing (fast path)
# ---------------------------------------------------------------------------


def _pack_idx16(flat):
    """[n] int64 row ids -> [128, n//16] int16 dma_gather index layout."""
    n = flat.shape[0]
    assert n % 128 == 0
    out = np.zeros((P, n // 16), np.uint16)
    cols = np.arange(n) // 16
    rows = np.arange(n) % 16
    for g in range(8):
        out[g * 16 + rows, cols] = flat.astype(np.uint16)
    return out.view(np.int16)


def _prep_fast(row, col, n):
    row = np.asarray(row).astype(np.int64)
    col = np.asarray(col).astype(np.int64)
    E = row.shape[0]
    NPC = -(-n // NC)
    R = -(-NPC // P) * P
    W = R // P
    NCHUNK = int(os.environ.get("GG_NCHUNK", "3"))
    NCHUNK = max(1, min(NCHUNK, W))
    CHB = [round(i * W / NCHUNK) * P for i in range(NCHUNK + 1)]  # row bounds

    deg = np.bincount(row, minlength=n)
    perm_pos = np.empty(n, np.int64)
    node_core = np.minimum(np.arange(n) // NPC, NC - 1)
    orders = []
    CWc = np.zeros((NC, W), np.int64)
    for c in range(NC):
        lo, hi = c * NPC, min((c + 1) * NPC, n)
        dd = deg[lo:hi]
        order = np.argsort(-dd, kind="stable")
        pos = np.empty(hi - lo, np.int64)
        pos[order] = np.arange(hi - lo)
        perm_pos[lo:hi] = pos
        orders.append(order + lo)
        sd = np.concatenate([dd[order], np.zeros(R - (hi - lo), np.int64)])
        CWc[c] = sd.reshape(W, P).max(axis=1)
    CW = CWc.max(axis=0).astype(np.int64)
    OWS = np.zeros(W + 1, np.int64)
    OWS[1:] = np.cumsum(CW)
    SC = int(OWS[-1])

    # chunk-major table row: rows of chunk k from all cores are contiguous
    pp = perm_pos
    chb = np.asarray(CHB, np.int64)
    kk = np.searchsorted(chb, pp, side="right") - 1
    chr_k = chb[kk + 1] - chb[kk]                    # rows in node's chunk
    tabrow = NC * chb[kk] + node_core * chr_k + (pp - chb[kk])

    ecore = np.minimum(row // NPC, NC - 1)
    lr = perm_pos[row]
    wv = lr // P
    rv = lr % P
    okey = ecore * R + lr
    eorder = np.argsort(okey, kind="stable")
    sk = okey[eorder]
    starts = np.r_[0, np.flatnonzero(np.diff(sk)) + 1]
    grp = np.zeros(E, np.int64)
    grp[starts] = 1
    grp = np.cumsum(grp) - 1
    jj = np.arange(E) - starts[grp]

    ec = ecore[eorder]
    flat = (OWS[wv[eorder]] + jj) * P + rv[eorder]
    tr = tabrow[col[eorder]]

    idx_all = np.zeros((NC, SC * P), np.int64)
    mL_all = np.zeros((NC, P, SC), np.float32)
    mR_all = np.zeros((NC, P, SC), np.float32)
    idx_all[ec, flat] = tr // 2
    left = (tr % 2 == 0).astype(np.float32)
    cidx = flat // P
    ridx = flat % P
    mL_all[ec, ridx, cidx] = left
    mR_all[ec, ridx, cidx] = 1.0 - left

    idx16 = [_pack_idx16(idx_all[c]) for c in range(NC)]

    return dict(
        R=R, W=W, NPC=NPC, NCHUNK=NCHUNK, CHB=tuple(CHB),
        CW=tuple(int(v) for v in CW), OWS=OWS, SC=SC,
        idx16=idx16, mL=[np.ascontiguousarray(mL_all[c]) for c in range(NC)],
        mR=[np.ascontiguousarray(mR_all[c]) for c in range(NC)],
        orders=orders,
    )


# ---------------------------------------------------------------------------
# bass program (fast path)
# ---------------------------------------------------------------------------


def _build_fast(R, W, CW, CHB, ln_trivial, b_zero,
                gsplit=64):
    import concourse.bass as bass  # noqa: F401
    import concourse.bacc as bacc
    import concourse.mybir as mybir
    import concourse.tile as tile
    from concourse.masks import make_identity

    F32 = mybir.dt.float32
    F16 = mybir.dt.float16
    I16 = mybir.dt.int16
    AF = mybir.ActivationFunctionType
    OP = mybir.AluOpType

    NPAD = NC * R
    OWS = [0]
    for c in CW:
        OWS.append(OWS[-1] + c)
    SC = OWS[-1]
    CWmax = max(CW)
    RG = [list(range(NC))]

    nc = bacc.Bacc("TRN2", target_bir_lowering=False)

    x_in = nc.dram_tensor("x", [R, D], F32, kind="ExternalInput")
    w0_in = nc.dram_tensor("W0", [D, D], F32, kind="ExternalInput")
    w1_in = nc.dram_tensor("W1", [D, D], F32, kind="ExternalInput")
    b0_in = nc.dram_tensor("b0", [1, D], F32, kind="ExternalInput")
    b1_in = nc.dram_tensor("b1", [1, D], F32, kind="ExternalInput")
    idx_in = nc.dram_tensor("idx16", [P, SC * 8], I16, kind="ExternalInput")
    ml_in = nc.dram_tensor("mL", [P, SC], F32, kind="ExternalInput")
    mr_in = nc.dram_tensor("mR", [P, SC], F32, kind="ExternalInput")
    lng_in = nc.dram_tensor("lng", [2, D], F32, kind="ExternalInput")
    lnb_in = nc.dram_tensor("lnb", [2, D], F32, kind="ExternalInput")
    out_t = nc.dram_tensor("out", [R, D], F32, kind="ExternalOutput")

    TAB = [nc.dram_tensor(f"tab{i}", [NPAD, D], F32, kind="Internal",
                          addr_space="Shared") for i in range(2)]
    con = [nc.dram_tensor(f"con{i}", [R, D], F32, kind="Internal")
           for i in range(2)]

    with tile.TileContext(nc) as tc, ExitStack() as ctx:
        singles = ctx.enter_context(tc.tile_pool(name="singles", bufs=1))
        gpool = ctx.enter_context(tc.tile_pool(name="gpool", bufs=4))
        spool = ctx.enter_context(tc.tile_pool(name="spool", bufs=3))
        wpool = ctx.enter_context(tc.tile_pool(name="wpool", bufs=3))
        dpool = ctx.enter_context(tc.tile_pool(name="dpool", bufs=6))
        hpool = ctx.enter_context(tc.tile_pool(name="hpool", bufs=3))
        psTR = ctx.enter_context(tc.tile_pool(name="psTR", bufs=2, space="PSUM"))
        psS = ctx.enter_context(tc.tile_pool(name="psS", bufs=2, space="PSUM"))
        psAG = ctx.enter_context(tc.tile_pool(name="psAG", bufs=2, space="PSUM"))
        psA2 = ctx.enter_context(tc.tile_pool(name="psA2", bufs=2, space="PSUM"))

        ident = singles.tile([P, P], F32)
        make_identity(nc, ident[:])
        ident16 = singles.tile([P, P], F16)
        nc.vector.tensor_copy(ident16[:], ident[:])

        _consts = {}

        def constcol(val):
            if val not in _consts:
                t = singles.tile([P, 1], F32, tag=f"const{len(_consts)}")
                nc.vector.memset(t[:], float(val))
                _consts[val] = t
            return _consts[val][:]

        w16 = []
        for t_in in (w0_in, w1_in):
            t32 = spool.tile([D, D], F32, tag="wld")
            nc.sync.dma_start(t32[:], t_in[:, :])
            t16 = singles.tile([D, D], F16, tag=f"w16{len(w16)}")
            nc.vector.tensor_copy(t16[:], t32[:])
            w16.append(t16)
        b_sb = [None, None]
        if not b_zero:
            for i, t_in in enumerate((b0_in, b1_in)):
                t = singles.tile([P, D], F32, tag=f"b{i}")
                nc.gpsimd.dma_start(t[:], t_in[0:1, :].to_broadcast([P, D]))
                b_sb[i] = t
        lng_sb = [None, None]
        lnb_sb = [None, None]
        if not ln_trivial:
            for i in range(2):
                g = singles.tile([P, D], F32, tag=f"lng{i}")
                nc.gpsimd.dma_start(g[:], lng_in[i:i + 1, :].to_broadcast([P, D]))
                lng_sb[i] = g
                b = singles.tile([P, D], F32, tag=f"lnb{i}")
                nc.gpsimd.dma_start(b[:], lnb_in[i:i + 1, :].to_broadcast([P, D]))
                lnb_sb[i] = b

        idx_sb = singles.tile([P, SC * 8], I16)
        nc.sync.dma_start(idx_sb[:], idx_in[:, :])
        mL_sb = singles.tile([P, SC], F32)
        nc.sync.dma_start(mL_sb[:], ml_in[:, :])
        mR_sb = singles.tile([P, SC], F32)
        nc.sync.dma_start(mR_sb[:], mr_in[:, :])

        def node_emit(h_sb, w, lnext):
            """h_sb: [P, D] f32 post-activation rows of window w.
            Publish hn16/s16 to con[lnext%2], s f32 to sscr[lnext%2]."""
            dst = con[lnext % 2]
            ss = wpool.tile([P, 1], F32, tag="nss")
            scr = spool.tile([P, D], F32, tag="nscr")
            nc.vector.scalar_tensor_tensor(
                out=scr[:], in0=h_sb[:], scalar=1.0, in1=h_sb[:],
                op0=OP.mult, op1=OP.mult, accum_out=ss[:])
            nc.scalar.activation(out=ss[:], in_=ss[:], func=AF.Ln,
                                 bias=constcol(1e-30))
            nc.scalar.activation(out=ss[:], in_=ss[:], func=AF.Exp,
                                 scale=-0.5)
            hn16 = spool.tile([P, D], F16, tag="hn16")
            nc.vector.tensor_scalar_mul(hn16[:], h_sb[:], ss[:])
            nc.sync.dma_start(
                dst[w * P:(w + 1) * P, :D // 2].bitcast(F16), hn16[:])
            hT_ps = psTR.tile([P, P], F32, tag="tr")
            nc.tensor.transpose(out=hT_ps[:], in_=h_sb[:], identity=ident[:])
            hT16 = spool.tile([P, D], F16, tag="hT16")
            nc.scalar.copy(hT16[:], hT_ps[:])
            s_ps = psS.tile([P, P], F32, tag="sps")
            nc.tensor.matmul(out=s_ps[:], lhsT=hT16[:],
                             rhs=w16[0 if lnext == 0 else 1][:],
                             start=True, stop=True)
            s16 = spool.tile([P, D], F16, tag="s16")
            nc.scalar.copy(s16[:], s_ps[:])
            nc.sync.dma_start(
                dst[w * P:(w + 1) * P, D // 2:D].bitcast(F16), s16[:])

        chend = {CHB[k + 1]: k for k in range(len(CHB) - 1)}

        def emit_chunk_cc(layer, w):
            k = chend.get((w + 1) * P)
            if k is None:
                return
            li = layer % 2
            lo, hi = CHB[k], CHB[k + 1]
            nc.gpsimd.collective_compute(
                "AllGather", OP.bypass, replica_groups=RG,
                ins=[con[li][lo:hi, :]],
                outs=[TAB[li][NC * lo:NC * hi, :]])

        # layer 0 node pass
        for w in range(W):
            x_sb = hpool.tile([P, D], F32, tag="x0")
            nc.sync.dma_start(x_sb[:], x_in[w * P:(w + 1) * P, :])
            node_emit(x_sb, w, 0)
            emit_chunk_cc(0, w)

        for layer in range(3):
            li = layer % 2
            tabv = TAB[li][:, :].rearrange("(a b) d -> a (b d)", b=2)
            for w in range(W):
                C = CW[w]
                ow = OWS[w]
                hs = spool.tile([P, 2 * D], F16, tag="hs")
                nc.sync.dma_start(hs[:],
                                  con[li][w * P:(w + 1) * P, :].bitcast(F16))
                sloc = hs[:, D:2 * D]
                if C == 0:
                    h2 = hpool.tile([P, D], F32, tag="h2")
                    nc.vector.tensor_scalar_mul(h2[:], sloc,
                                                float(np.exp(1.0)))
                else:
                    sC32 = gpool.tile([P, CWmax, 2 * D], F32, tag="sC")
                    for t0 in range(0, C, gsplit):
                        t1 = min(t0 + gsplit, C)
                        nc.gpsimd.dma_gather(
                            out_ap=sC32[:, t0:t1, :],
                            in_ap=tabv,
                            idxs_ap=idx_sb[:, (ow + t0) * 8:(ow + t1) * 8],
                            num_idxs=(t1 - t0) * P,
                            num_idxs_reg=(t1 - t0) * P,
                            elem_size=2 * D)
                    sC = sC32[:, :, :].bitcast(F16)

                    simL = wpool.tile([P, CWmax], F32, tag="simL")
                    simR = wpool.tile([P, CWmax], F32, tag="simR")
                    scrA = spool.tile([P, D], F16, tag="scrA")
                    scrB = spool.tile([P, D], F16, tag="scrB")
                    for c in range(C):
                        nc.vector.scalar_tensor_tensor(
                            out=scrA[:], in0=sC[:, c, 0:D],
                            scalar=mL_sb[:, ow + c:ow + c + 1], in1=hs[:, 0:D],
                            op0=OP.mult, op1=OP.mult,
                            accum_out=simL[:, c:c + 1])
                        nc.vector.scalar_tensor_tensor(
                            out=scrB[:], in0=sC[:, c, 2 * D:3 * D],
                            scalar=mR_sb[:, ow + c:ow + c + 1], in1=hs[:, 0:D],
                            op0=OP.mult, op1=OP.mult,
                            accum_out=simR[:, c:c + 1])
                    sims = wpool.tile([P, CWmax], F32, tag="sims")
                    nc.vector.tensor_tensor(out=sims[:, :C], in0=simL[:, :C],
                                            in1=simR[:, :C], op=OP.add)
                    thr = wpool.tile([P, CWmax], F32, tag="thr")
                    nc.vector.tensor_scalar(out=thr[:, :C], in0=sims[:, :C],
                                            scalar1=0.1, scalar2=None,
                                            op0=OP.is_ge)
                    nc.vector.tensor_tensor(out=sims[:, :C], in0=sims[:, :C],
                                            in1=thr[:, :C], op=OP.mult)
                    rs = wpool.tile([P, 1], F32, tag="rs")
                    nc.vector.tensor_reduce(out=rs[:], in_=sims[:, :C],
                                            axis=mybir.AxisListType.X,
                                            op=OP.add)
                    deg = wpool.tile([P, 1], F32, tag="deg")
                    nc.vector.tensor_reduce(out=deg[:], in_=thr[:, :C],
                                            axis=mybir.AxisListType.X,
                                            op=OP.add)
                    # guarded 1/rs
                    g01 = wpool.tile([P, 1], F32, tag="g01")
                    nc.vector.tensor_scalar(out=g01[:], in0=rs[:], scalar1=0.0,
                                            scalar2=None, op0=OP.is_gt)
                    nc.vector.scalar_tensor_tensor(
                        out=rs[:], in0=rs[:], scalar=1.0, in1=g01[:],
                        op0=OP.subtract, op1=OP.mult)
                    nc.vector.tensor_scalar_add(rs[:], rs[:], 1.0)
                    nc.vector.reciprocal(rs[:], rs[:])
                    att = wpool.tile([P, CWmax], F32, tag="att")
                    nc.vector.tensor_scalar(out=att[:, :C], in0=sims[:, :C],
                                            scalar1=rs[:], scalar2=None,
                                            op0=OP.mult)
                    we = wpool.tile([P, CWmax], F32, tag="we")
                    nc.scalar.activation(out=we[:, :C], in_=att[:, :C],
                                         func=AF.Exp)
                    nc.vector.tensor_tensor(out=we[:, :C], in0=we[:, :C],
                                            in1=thr[:, :C], op=OP.mult)
                    wL = wpool.tile([P, CWmax], F32, tag="wL")
                    nc.vector.tensor_tensor(out=wL[:, :C], in0=we[:, :C],
                                            in1=mL_sb[:, ow:ow + C],
                                            op=OP.mult)
                    wR = wpool.tile([P, CWmax], F32, tag="wR")
                    nc.vector.tensor_tensor(out=wR[:, :C], in0=we[:, :C],
                                            in1=wL[:, :C], op=OP.subtract)
                    # lam / w_diag
                    wd = wpool.tile([P, 1], F32, tag="wd")
                    nc.vector.tensor_scalar_add(wd[:], deg[:], 1.0)
                    nc.vector.reciprocal(wd[:], wd[:])
                    nc.scalar.activation(out=wd[:], in_=wd[:], func=AF.Exp)
                    # agg^T via PE: sum_c s_half^T(lhsT) x diag(w)
                    aggT_ps = psAG.tile([P, P], F32, tag="aggT")
                    for c in range(C):
                        dL = dpool.tile([P, P], F16, tag="dL")
                        nc.vector.tensor_scalar(
                            out=dL[:], in0=ident16[:],
                            scalar1=wL[:, c:c + 1], scalar2=None, op0=OP.mult)
                        nc.tensor.matmul(out=aggT_ps[:],
                                         lhsT=sC[:, c, D:2 * D], rhs=dL[:],
                                         start=(c == 0), stop=False)
                        dR = dpool.tile([P, P], F16, tag="dR")
                        nc.scalar.activation(out=dR[:], in_=ident16[:],
                                             func=AF.Identity,
                                             scale=wR[:, c:c + 1])
                        nc.tensor.matmul(out=aggT_ps[:],
                                         lhsT=sC[:, c, 3 * D:4 * D], rhs=dR[:],
                                         start=False, stop=(c == C - 1))
                    aggT_sb = spool.tile([P, D], F32, tag="aggT_sb")
                    nc.scalar.copy(aggT_sb[:], aggT_ps[:])
                    agg_ps = psA2.tile([P, P], F32, tag="agg")
                    nc.tensor.transpose(out=agg_ps[:], in_=aggT_sb[:],
                                        identity=ident[:])
                    h2 = hpool.tile([P, D], F32, tag="h2")
                    nc.vector.scalar_tensor_tensor(
                        out=h2[:], in0=sloc, scalar=wd[:], in1=agg_ps[:],
                        op0=OP.mult, op1=OP.add)
                if not b_zero:
                    nc.vector.tensor_tensor(
                        out=h2[:], in0=h2[:],
                        in1=b_sb[0 if layer == 0 else 1][:], op=OP.add)
                if layer < 2:
                    st6 = wpool.tile([P, 6], F32, tag="st6")
                    nc.vector.bn_stats(out=st6[:], in_=h2[:])
                    mv = wpool.tile([P, 2], F32, tag="mv")
                    nc.vector.bn_aggr(out=mv[:], in_=st6[:])
                    sd = wpool.tile([P, 1], F32, tag="sd")
                    nc.scalar.activation(out=sd[:], in_=mv[:, 1:2],
                                         func=AF.Ln, bias=constcol(EPS))
                    nc.scalar.activation(out=sd[:], in_=sd[:],
                                         func=AF.Exp, scale=-0.5)
                    if ln_trivial:
                        # relu((h - mu) * isd) in one Act op
                        nb = wpool.tile([P, 1], F32, tag="nb")
                        nc.vector.tensor_scalar(
                            out=nb[:], in0=mv[:, 0:1], scalar1=sd[:],
                            scalar2=-1.0, op0=OP.mult, op1=OP.mult)
                        nc.scalar.activation(out=h2[:], in_=h2[:],
                                             func=AF.Relu, bias=nb[:],
                                             scale=sd[:])
                    else:
                        nc.vector.tensor_scalar(
                            out=h2[:], in0=h2[:], scalar1=mv[:, 0:1],
                            scalar2=sd[:], op0=OP.subtract, op1=OP.mult)
                        nc.vector.tensor_tensor(out=h2[:], in0=h2[:],
                                                in1=lng_sb[layer][:],
                                                op=OP.mult)
                        nc.vector.tensor_tensor(out=h2[:], in0=h2[:],
                                                in1=lnb_sb[layer][:],
                                                op=OP.add)
                        nc.vector.tensor_scalar(out=h2[:], in0=h2[:],
                                                scalar1=0.0, scalar2=None,
                                                op0=OP.max)
                    node_emit(h2, w, layer + 1)
                    emit_chunk_cc(layer + 1, w)
                else:
                    mx = wpool.tile([P, 1], F32, tag="mx")
                    nc.vector.tensor_reduce(out=mx[:], in_=h2[:],
                                            axis=mybir.AxisListType.X,
                                            op=OP.max)
                    nc.vector.tensor_scalar_mul(mx[:], mx[:], -1.0)
                    ex = spool.tile([P, D], F32, tag="ex")
                    se = wpool.tile([P, 1], F32, tag="se")
                    nc.scalar.activation(out=ex[:], in_=h2[:], func=AF.Exp,
                                         bias=mx[:], accum_out=se[:])
                    nc.scalar.activation(out=se[:], in_=se[:], func=AF.Ln)
                    nc.vector.tensor_tensor(out=mx[:], in0=mx[:], in1=se[:],
                                            op=OP.subtract)
                    nc.vector.tensor_scalar_add(h2[:], h2[:], mx[:])
                    nc.sync.dma_start(out_t[w * P:(w + 1) * P, :], h2[:])

    nc.compile()
    return nc


# ---------------------------------------------------------------------------
# fast-path entry helpers
# ---------------------------------------------------------------------------

_CACHE_FAST = {}


def _kernel_fast(inputs, prep, ln_trivial, b_zero):
    from concourse.bass_utils import run_bass_kernel_spmd

    R, W = prep["R"], prep["W"]
    gsplit = int(os.environ.get("GG_GSPLIT", "0")) or 6
    key = (R, W, prep["CW"], prep["CHB"], ln_trivial, b_zero, gsplit)
    if key not in _CACHE_FAST:
        _CACHE_FAST[key] = _build_fast(R, W, prep["CW"], prep["CHB"],
                                       ln_trivial, b_zero, gsplit)
    nc = _CACHE_FAST[key]

    x = np.ascontiguousarray(np.asarray(inputs["x"], dtype=np.float32))
    n = x.shape[0]
    lng = np.stack([np.asarray(inputs["ln1_g"], np.float32),
                    np.asarray(inputs["ln2_g"], np.float32)])
    lnb = np.stack([np.asarray(inputs["ln1_b"], np.float32),
                    np.asarray(inputs["ln2_b"], np.float32)])
    in_maps = []
    for c in range(NC):
        order = prep["orders"][c]
        xp = np.zeros((R, D), np.float32)
        xp[:order.shape[0]] = x[order]
        in_maps.append({
            "x": xp,
            "W0": np.ascontiguousarray(np.asarray(inputs["W0"], np.float32)),
            "W1": np.ascontiguousarray(np.asarray(inputs["W1"], np.float32)),
            "b0": np.asarray(inputs["b0"], np.float32).reshape(1, D).copy(),
            "b1": np.asarray(inputs["b1"], np.float32).reshape(1, D).copy(),
            "idx16": prep["idx16"][c],
            "mL": prep["mL"][c], "mR": prep["mR"][c],
            "lng": np.ascontiguousarray(lng), "lnb": np.ascontiguousarray(lnb),
        })
    res = run_bass_kernel_spmd(nc, in_maps, core_ids=list(range(NC)),
                               trace=bool(int(os.environ.get("GG_TRACE", "0"))))
    out = np.empty((n, D), np.float32)
    for c in range(NC):
        order = prep["orders"][c]
        out[order] = res.results[c]["out"][:order.shape[0]]
    if os.environ.get("GG_RESULT_OBJ"):
        kernel._last_results = res
    return out


# ===========================================================================
# general fallback path (original implementation, unchanged)
# ===========================================================================

SW = 192          # s-table row width (s[128] | rs | pad)


def _preprocess(row, col, n_nodes):
    row = np.asarray(row).astype(np.int64)
    col = np.asarray(col).astype(np.int64)
    E = row.shape[0]
    R = int(np.ceil(n_nodes / NC / P)) * P
    W = R // P
    NPAD = NC * R

    keys = np.sort(row * n_nodes + col)
    rkeys = col * n_nodes + row
    pos = np.clip(np.searchsorted(keys, rkeys), 0, E - 1)
    has_rev_e = (keys[pos] == rkeys).astype(np.float32)

    order = np.lexsort((col, row))
    srow, scol, shrev = row[order], col[order], has_rev_e[order]

    chunk = srow // R
    lr = srow - chunk * R
    win = lr // P
    rel = lr % P
    gw = chunk * W + win
    cnt = np.bincount(gw, minlength=NC * W)
    K = max(1, int(np.ceil(cnt.max() / P)))
    S = K * P

    starts = np.zeros(NC * W, dtype=np.int64)
    starts[1:] = np.cumsum(cnt)[:-1]
    slot = gw * S + (np.arange(E) - starts[gw])

    colid = np.zeros(NC * W * S, np.int64)       # pads -> row 0 (+ vmask)
    relc = np.full(NC * W * S, P - 1, np.float32)
    hrev = np.zeros(NC * W * S, np.float32)
    vmask = np.zeros(NC * W * S, np.float32)
    mleft = np.ones(NC * W * S, np.float32)

    colid[slot] = scol // 2                       # pair-packed row id
    mleft[slot] = (scol % 2 == 0).astype(np.float32)
    relc[slot] = rel
    hrev[slot] = shrev
    vmask[slot] = 1.0

    def per_core_pk(arr):
        a = arr.reshape(NC, W, K, P)
        return [np.ascontiguousarray(a[c].transpose(2, 0, 1).reshape(P, W * K))
                for c in range(NC)]

    idx16 = [np.concatenate(
        [_pack_idx16(colid[(c * W + w) * S:(c * W + w + 1) * S])
         for w in range(W)], axis=1) for c in range(NC)]

    return dict(
        R=R, W=W, K=K, S=S, NPAD=NPAD, E=E,
        idx16=idx16, relc=per_core_pk(relc), hrev=per_core_pk(hrev),
        vmask=per_core_pk(vmask), mleft=per_core_pk(mleft),
    )


def _build(R, W, K, wd0, wd1, bd, ln_trivial, b_zero):
    import concourse.bass as bass  # noqa: F401
    import concourse.bacc as bacc
    import concourse.mybir as mybir
    import concourse.tile as tile
    from concourse.masks import make_identity

    F32 = mybir.dt.float32
    I16 = mybir.dt.int16
    AF = mybir.ActivationFunctionType
    OP = mybir.AluOpType

    S = K * P
    NPAD = NC * R
    NRS = NC * R
    RG = [list(range(NC))]
    SC = S // 16                     # idx16 columns per window

    nc = bacc.Bacc("TRN2", target_bir_lowering=False)

    x_in = nc.dram_tensor("x", [R, D], F32, kind="ExternalInput")
    w0_in = nc.dram_tensor("W0", [D, D], F32, kind="ExternalInput")
    w1_in = nc.dram_tensor("W1", [D, D], F32, kind="ExternalInput")
    b0_in = nc.dram_tensor("b0", [1, D], F32, kind="ExternalInput")
    b1_in = nc.dram_tensor("b1", [1, D], F32, kind="ExternalInput")
    idx_in = nc.dram_tensor("idx16", [P, W * SC], I16, kind="ExternalInput")
    relc_in = nc.dram_tensor("relc", [P, W * K], F32, kind="ExternalInput")
    hrev_in = nc.dram_tensor("hrev", [P, W * K], F32, kind="ExternalInput")
    vmask_in = nc.dram_tensor("vmask", [P, W * K], F32, kind="ExternalInput")
    mleft_in = nc.dram_tensor("mleft", [P, W * K], F32, kind="ExternalInput")
    lng_in = nc.dram_tensor("lng", [2, D], F32, kind="ExternalInput")
    lnb_in = nc.dram_tensor("lnb", [2, D], F32, kind="ExternalInput")
    out_t = nc.dram_tensor("out", [R, D], F32, kind="ExternalOutput")

    TABH = nc.dram_tensor("tabh", [NPAD, D], F32, kind="Internal",
                          addr_space="Shared")
    TABS = nc.dram_tensor("tabs", [NPAD, SW], F32, kind="Internal",
                          addr_space="Shared")
    rs_tab = nc.dram_tensor("rstab", [NRS, 1], F32, kind="Internal",
                            addr_space="Shared")
    con_h = [nc.dram_tensor(f"conh{i}", [R, D], F32, kind="Internal")
             for i in range(2)]
    con_s = [nc.dram_tensor(f"cons{i}", [R, SW], F32, kind="Internal")
             for i in range(2)]
    rs_con = nc.dram_tensor("rscon", [W, P], F32, kind="Internal")
    rden_d = nc.dram_tensor("rdend", [W, P], F32, kind="Internal")

    with tile.TileContext(nc) as tc, ExitStack() as ctx:
        singles = ctx.enter_context(tc.tile_pool(name="singles", bufs=1))
        hpool = ctx.enter_context(tc.tile_pool(name="hpool", bufs=3))
        gpool = ctx.enter_context(tc.tile_pool(name="gpool", bufs=4))
        ipool = ctx.enter_context(tc.tile_pool(name="ipool", bufs=2))
        spool = ctx.enter_context(tc.tile_pool(name="spool", bufs=3))
        wpool = ctx.enter_context(tc.tile_pool(name="wpool", bufs=4))
        psTR = ctx.enter_context(tc.tile_pool(name="psTR", bufs=1, space="PSUM"))
        psIT = ctx.enter_context(tc.tile_pool(name="psIT", bufs=2, space="PSUM"))
        psHR = ctx.enter_context(tc.tile_pool(name="psHR", bufs=2, space="PSUM"))
        psAG = ctx.enter_context(tc.tile_pool(name="psAG", bufs=1, space="PSUM"))
        psSM = ctx.enter_context(tc.tile_pool(name="psSM", bufs=1, space="PSUM"))

        ident = singles.tile([P, P], F32)
        make_identity(nc, ident[:])
        iota = singles.tile([P, P], mybir.dt.int32)
        nc.gpsimd.iota(iota[:], pattern=[[1, P]], base=0, channel_multiplier=0)
        iota_f = singles.tile([P, P], F32)
        nc.vector.tensor_copy(iota_f[:], iota[:])

        _consts = {}

        def constcol(val):
            if val not in _consts:
                t = singles.tile([P, 1], F32, tag=f"const{len(_consts)}")
                nc.vector.memset(t[:], float(val))
                _consts[val] = t
            return _consts[val][:]

        w0_sb = singles.tile([D, D], F32)
        nc.sync.dma_start(w0_sb[:], w0_in[:, :])
        w1_sb = singles.tile([D, D], F32)
        nc.sync.dma_start(w1_sb[:], w1_in[:, :])
        b_sb = []
        for t_in in (b0_in, b1_in):
            t = singles.tile([P, D], F32)
            nc.gpsimd.dma_start(t[:], t_in[0:1, :].to_broadcast([P, D]))
            b_sb.append(t)
        lng_sb = [None, None]
        lnb_sb = [None, None]
        if not ln_trivial:
            for i in range(2):
                g = singles.tile([P, D], F32, tag=f"lng{i}")
                nc.gpsimd.dma_start(g[:], lng_in[i:i + 1, :].to_broadcast([P, D]))
                lng_sb[i] = g
                b = singles.tile([P, D], F32, tag=f"lnb{i}")
                nc.gpsimd.dma_start(b[:], lnb_in[i:i + 1, :].to_broadcast([P, D]))
                lnb_sb[i] = b

        idx_sb = singles.tile([P, W * SC], I16)
        nc.sync.dma_start(idx_sb[:], idx_in[:, :])
        relc_sb = singles.tile([P, W * K], F32)
        nc.sync.dma_start(relc_sb[:], relc_in[:, :])
        hrev_sb = singles.tile([P, W * K], F32)
        nc.sync.dma_start(hrev_sb[:], hrev_in[:, :])
        vmask_sb = singles.tile([P, W * K], F32)
        nc.sync.dma_start(vmask_sb[:], vmask_in[:, :])
        mleft_sb = singles.tile([P, W * K], F32)
        nc.sync.dma_start(mleft_sb[:], mleft_in[:, :])

        sims = singles.tile([P, W * K], F32)

        zpad = singles.tile([P, SW - D], F32)
        nc.vector.memset(zpad[:], 0.0)
        for ci in range(2):
            for w in range(W):
                nc.sync.dma_start(con_s[ci][w * P:(w + 1) * P, D:], zpad[:])

        def node_ops(h_sb, w, layer_next):
            dsth = con_h[layer_next % 2]
            dsts = con_s[layer_next % 2]
            wmat = w0_sb if layer_next == 0 else w1_sb
            ss = wpool.tile([P, 1], F32, tag="ss")
            scr = spool.tile([P, D], F32, tag="nscr")
            nc.vector.scalar_tensor_tensor(
                out=scr[:], in0=h_sb[:], scalar=1.0, in1=h_sb[:],
                op0=OP.mult, op1=OP.mult, accum_out=ss[:])
            nc.scalar.activation(out=ss[:], in_=ss[:], func=AF.Sqrt,
                                 bias=constcol(1e-30))
            nc.vector.reciprocal(ss[:], ss[:])
            hn = spool.tile([P, D], F32, tag="hn")
            nc.vector.tensor_scalar_mul(hn[:], h_sb[:], ss[:])
            nc.sync.dma_start(dsth[w * P:(w + 1) * P, :], hn[:])
            hT_ps = psTR.tile([P, P], F32, tag="tr")
            nc.tensor.transpose(out=hT_ps[:], in_=h_sb[:], identity=ident[:])
            hT = spool.tile([P, D], F32, tag="hT")
            nc.scalar.copy(hT[:], hT_ps[:])
            s_ps = psTR.tile([P, P], F32, tag="tr")
            nc.tensor.matmul(out=s_ps[:], lhsT=hT[:], rhs=wmat[:],
                             start=True, stop=True)
            s_sb = spool.tile([P, D], F32, tag="s_sb")
            nc.scalar.copy(s_sb[:], s_ps[:])
            nc.sync.dma_start(dsts[w * P:(w + 1) * P, :D], s_sb[:])

        for w in range(W):
            h_sb = hpool.tile([P, D], F32, tag="h0")
            nc.sync.dma_start(h_sb[:], x_in[w * P:(w + 1) * P, :])
            node_ops(h_sb, w, 0)

        for layer in range(3):
            ch = con_h[layer % 2]
            cs = con_s[layer % 2]
            bias = b_sb[0] if layer == 0 else b_sb[1]

            nc.gpsimd.collective_compute(
                "AllGather", OP.bypass, replica_groups=RG,
                ins=[ch[:, :]], outs=[TABH[:NPAD, :]])
            nc.gpsimd.collective_compute(
                "AllGather", OP.bypass, replica_groups=RG,
                ins=[cs[:, :]], outs=[TABS[:NPAD, :]])

            # ---------- B1: sims + rs ----------
            for w in range(W):
                hnC = gpool.tile([P, K, 2 * D], F32, tag="hnC")
                for t0 in range(0, K, 6):
                    t1 = min(t0 + 6, K)
                    nc.gpsimd.dma_gather(
                        out_ap=hnC[:, t0:t1, :],
                        in_ap=TABH[:, :].rearrange("(a b) d -> a (b d)", b=2),
                        idxs_ap=idx_sb[:, w * SC + t0 * 8:w * SC + t1 * 8],
                        num_idxs=(t1 - t0) * P, num_idxs_reg=(t1 - t0) * P,
                        elem_size=2 * D)
                hnW = wpool.tile([P, D], F32, tag="hnW")
                nc.sync.dma_start(hnW[:], ch[w * P:(w + 1) * P, :])
                I_w = ipool.tile([P, S], F32, tag="I_w")
                simL = wpool.tile([P, K], F32, tag="simL")
                simR = wpool.tile([P, K], F32, tag="simR")
                for t in range(K):
                    c0 = w * K + t
                    nc.vector.tensor_scalar(
                        out=I_w[:, t * P:(t + 1) * P], in0=iota_f[:],
                        scalar1=relc_sb[:, c0:c0 + 1], scalar2=None,
                        op0=OP.is_equal)
                    IT_ps = psIT.tile([P, P], F32, tag="IT")
                    nc.tensor.transpose(out=IT_ps[:],
                                        in_=I_w[:, t * P:(t + 1) * P],
                                        identity=ident[:])
                    IT = wpool.tile([P, P], F32, tag="ITsb")
                    nc.scalar.copy(IT[:], IT_ps[:])
                    hre_ps = psHR.tile([P, P], F32, tag="hre")
                    nc.tensor.matmul(out=hre_ps[:], lhsT=IT[:], rhs=hnW[:],
                                     start=True, stop=True)
                    scr = spool.tile([P, D], F32, tag="simscr")
                    nc.vector.scalar_tensor_tensor(
                        out=scr[:], in0=hnC[:, t, :D], scalar=1.0,
                        in1=hre_ps[:], op0=OP.mult, op1=OP.mult,
                        accum_out=simL[:, t:t + 1])
                    nc.vector.scalar_tensor_tensor(
                        out=scr[:], in0=hnC[:, t, D:], scalar=1.0,
                        in1=hre_ps[:], op0=OP.mult, op1=OP.mult,
                        accum_out=simR[:, t:t + 1])
                cw = slice(w * K, (w + 1) * K)
                nc.vector.tensor_tensor(out=simL[:], in0=simL[:], in1=simR[:],
                                        op=OP.subtract)
                nc.vector.tensor_tensor(out=simL[:], in0=simL[:],
                                        in1=mleft_sb[:, cw], op=OP.mult)
                nc.vector.tensor_tensor(out=sims[:, cw], in0=simL[:],
                                        in1=simR[:], op=OP.add)
                thr = wpool.tile([P, K], F32, tag="thr")
                nc.vector.tensor_scalar(out=thr[:], in0=sims[:, cw],
                                        scalar1=0.1, scalar2=None, op0=OP.is_ge)
                nc.vector.tensor_tensor(out=thr[:], in0=thr[:],
                                        in1=vmask_sb[:, cw], op=OP.mult)
                nc.vector.tensor_tensor(out=sims[:, cw], in0=sims[:, cw],
                                        in1=thr[:], op=OP.mult)
                rs_ps = psSM.tile([1, P], F32, tag="rs")
                for t in range(K):
                    c0 = w * K + t
                    nc.tensor.matmul(out=rs_ps[:], lhsT=sims[:, c0:c0 + 1],
                                     rhs=I_w[:, t * P:(t + 1) * P],
                                     start=(t == 0), stop=(t == K - 1))
                rs_sb = wpool.tile([1, P], F32, tag="rs_sb")
                nc.scalar.copy(rs_sb[:], rs_ps[:])
                nc.sync.dma_start(rs_con[w:w + 1, :], rs_sb[:])

            nc.gpsimd.collective_compute(
                "AllGather", OP.bypass, replica_groups=RG,
                ins=[rs_con[:, :]], outs=[rs_tab[:NRS, :]])
            with nc.allow_non_contiguous_dma(reason="rs column scatter"):
                for ci in range(NC):
                    nc.sync.dma_start(
                        TABS[ci * R:(ci + 1) * R, D:D + 1],
                        rs_tab[ci * R:(ci + 1) * R, :])

            # ---------- B2: att, mask, conv ----------
            for w in range(W):
                cw = slice(w * K, (w + 1) * K)
                sC = gpool.tile([P, K, 2 * SW], F32, tag="sC")
                for t0 in range(0, K, 6):
                    t1 = min(t0 + 6, K)
                    nc.gpsimd.dma_gather(
                        out_ap=sC[:, t0:t1, :],
                        in_ap=TABS[:, :].rearrange("(a b) d -> a (b d)", b=2),
                        idxs_ap=idx_sb[:, w * SC + t0 * 8:w * SC + t1 * 8],
                        num_idxs=(t1 - t0) * P, num_idxs_reg=(t1 - t0) * P,
                        elem_size=2 * SW)
                rsr = wpool.tile([1, P], F32, tag="rsrow")
                nc.sync.dma_start(rsr[:], rs_con[w:w + 1, :])
                g01 = wpool.tile([1, P], F32, tag="g01")
                nc.vector.tensor_scalar(out=g01[:], in0=rsr[:], scalar1=0.0,
                                        scalar2=None, op0=OP.is_gt)
                nc.vector.scalar_tensor_tensor(
                    out=rsr[:], in0=rsr[:], scalar=1.0, in1=g01[:],
                    op0=OP.subtract, op1=OP.mult)
                nc.vector.tensor_scalar_add(rsr[:], rsr[:], 1.0)
                nc.vector.reciprocal(rsr[:], rsr[:])
                nc.sync.dma_start(rden_d[w:w + 1, :], rsr[:])
                rden_col = wpool.tile([P, 1], F32, tag="rdenc")
                nc.sync.dma_start(rden_col[:, :], rden_d[w, :, None])

                att = wpool.tile([P, K], F32, tag="att")
                rev = wpool.tile([P, K], F32, tag="rev")
                scr = wpool.tile([P, K], F32, tag="mscr")
                rde = wpool.tile([P, K], F32, tag="rde")
                for t in range(K):
                    c0 = w * K + t
                    I_t = ipool.tile([P, P], F32, tag="I_t")
                    nc.vector.tensor_scalar(
                        out=I_t[:], in0=iota_f[:],
                        scalar1=relc_sb[:, c0:c0 + 1], scalar2=None,
                        op0=OP.is_equal)
                    IT_ps = psIT.tile([P, P], F32, tag="IT")
                    nc.tensor.transpose(out=IT_ps[:], in_=I_t[:],
                                        identity=ident[:])
                    IT = wpool.tile([P, P], F32, tag="ITsb")
                    nc.scalar.copy(IT[:], IT_ps[:])
                    rex_ps = psHR.tile([P, P], F32, tag="hre")
                    nc.tensor.matmul(out=rex_ps[:, 0:1], lhsT=IT[:],
                                     rhs=rden_col[:], start=True, stop=True)
                    nc.scalar.copy(rde[:, t:t + 1], rex_ps[:, 0:1])
                nc.vector.tensor_tensor(out=att[:], in0=sims[:, cw],
                                        in1=rde[:], op=OP.mult)
                rs_c = wpool.tile([P, K], F32, tag="rs_c")
                nc.vector.tensor_tensor(out=rs_c[:], in0=sC[:, :, D],
                                        in1=sC[:, :, SW + D], op=OP.subtract)
                nc.vector.tensor_tensor(out=rs_c[:], in0=rs_c[:],
                                        in1=mleft_sb[:, cw], op=OP.mult)
                nc.vector.tensor_tensor(out=rs_c[:], in0=rs_c[:],
                                        in1=sC[:, :, SW + D], op=OP.add)
                nc.vector.tensor_scalar(out=scr[:], in0=rs_c[:], scalar1=0.0,
                                        scalar2=None, op0=OP.is_gt)
                nc.vector.scalar_tensor_tensor(
                    out=rev[:], in0=rs_c[:], scalar=1.0, in1=scr[:],
                    op0=OP.subtract, op1=OP.mult)
                nc.vector.tensor_scalar_add(rev[:], rev[:], 1.0)
                nc.vector.reciprocal(rev[:], rev[:])
                nc.vector.tensor_tensor(out=rev[:], in0=rev[:],
                                        in1=sims[:, cw], op=OP.mult)
                nc.vector.tensor_tensor(out=rev[:], in0=rev[:],
                                        in1=hrev_sb[:, cw], op=OP.mult)
                nc.scalar.activation(out=rev[:], in_=rev[:], func=AF.Identity,
                                     bias=constcol(bd), scale=wd1)
                nc.vector.scalar_tensor_tensor(
                    out=scr[:], in0=att[:], scalar=wd0, in1=rev[:],
                    op0=OP.mult, op1=OP.add)
                nc.vector.tensor_scalar(out=scr[:], in0=scr[:], scalar1=0.0,
                                        scalar2=None, op0=OP.is_gt)
                nc.vector.tensor_tensor(out=att[:], in0=att[:], in1=scr[:],
                                        op=OP.mult)
                nc.vector.tensor_scalar(out=scr[:], in0=att[:], scalar1=0.0,
                                        scalar2=None, op0=OP.not_equal)
                nc.scalar.activation(out=att[:], in_=att[:], func=AF.Exp)
                nc.vector.tensor_tensor(out=att[:], in0=att[:], in1=scr[:],
                                        op=OP.mult)
                attL = wpool.tile([P, K], F32, tag="attL")
                attR = wpool.tile([P, K], F32, tag="attR")
                nc.vector.tensor_tensor(out=attL[:], in0=att[:],
                                        in1=mleft_sb[:, cw], op=OP.mult)
                nc.vector.tensor_tensor(out=attR[:], in0=att[:],
                                        in1=attL[:], op=OP.subtract)
                agg_ps = psAG.tile([P, P + 1], F32, tag="agg")
                for t in range(K):
                    c0 = w * K + t
                    I_t = ipool.tile([P, P], F32, tag="I_t2")
                    nc.vector.tensor_scalar(
                        out=I_t[:], in0=iota_f[:],
                        scalar1=relc_sb[:, c0:c0 + 1], scalar2=None,
                        op0=OP.is_equal)
                    wsc = spool.tile([P, P + 1], F32, tag="wsc")
                    nc.vector.tensor_scalar_mul(
                        wsc[:, :D], sC[:, t, :D], attL[:, t:t + 1])
                    nc.vector.scalar_tensor_tensor(
                        out=wsc[:, :D], in0=sC[:, t, SW:SW + D],
                        scalar=attR[:, t:t + 1], in1=wsc[:, :D],
                        op0=OP.mult, op1=OP.add)
                    nc.vector.tensor_copy(wsc[:, D:D + 1], scr[:, t:t + 1])
                    nc.tensor.matmul(out=agg_ps[:], lhsT=I_t[:], rhs=wsc[:],
                                     start=(t == 0), stop=(t == K - 1))
                lam = wpool.tile([P, 1], F32, tag="lam")
                nc.vector.tensor_scalar_add(lam[:], agg_ps[:, D:D + 1], 1.0)
                nc.vector.reciprocal(lam[:], lam[:])
                nc.scalar.activation(out=lam[:], in_=lam[:], func=AF.Exp)
                s_loc = spool.tile([P, D], F32, tag="s_loc")
                nc.sync.dma_start(s_loc[:], cs[w * P:(w + 1) * P, :D])
                h2 = hpool.tile([P, D], F32, tag="h2")
                nc.vector.scalar_tensor_tensor(
                    out=h2[:], in0=s_loc[:], scalar=lam[:], in1=agg_ps[:, :D],
                    op0=OP.mult, op1=OP.add)
                if not b_zero:
                    nc.vector.tensor_tensor(out=h2[:], in0=h2[:], in1=bias[:],
                                            op=OP.add)
                if layer < 2:
                    st6 = wpool.tile([P, 6], F32, tag="st6")
                    nc.vector.bn_stats(out=st6[:], in_=h2[:])
                    mv = wpool.tile([P, 2], F32, tag="mv")
                    nc.vector.bn_aggr(out=mv[:], in_=st6[:])
                    sd = wpool.tile([P, 1], F32, tag="sd")
                    nc.scalar.activation(out=sd[:], in_=mv[:, 1:2],
                                         func=AF.Sqrt, bias=constcol(EPS))
                    nc.vector.reciprocal(sd[:], sd[:])
                    nc.vector.tensor_scalar(
                        out=h2[:], in0=h2[:], scalar1=mv[:, 0:1],
                        scalar2=sd[:], op0=OP.subtract, op1=OP.mult)
                    if not ln_trivial:
                        nc.vector.tensor_tensor(out=h2[:], in0=h2[:],
                                                in1=lng_sb[layer][:],
                                                op=OP.mult)
                        nc.vector.tensor_tensor(out=h2[:], in0=h2[:],
                                                in1=lnb_sb[layer][:],
                                                op=OP.add)
                    nc.scalar.activation(out=h2[:], in_=h2[:], func=AF.Relu)
                    node_ops(h2, w, layer + 1)
                else:
                    mx = wpool.tile([P, 1], F32, tag="mx")
                    nc.vector.tensor_reduce(out=mx[:], in_=h2[:],
                                            axis=mybir.AxisListType.X,
                                            op=OP.max)
                    nc.vector.tensor_scalar_mul(mx[:], mx[:], -1.0)
                    ex = spool.tile([P, D], F32, tag="ex")
                    se = wpool.tile([P, 1], F32, tag="se")
                    nc.scalar.activation(out=ex[:], in_=h2[:], func=AF.Exp,
                                         bias=mx[:], accum_out=se[:])
                    nc.scalar.activation(out=se[:], in_=se[:], func=AF.Ln)
                    nc.vector.tensor_tensor(out=mx[:], in0=mx[:], in1=se[:],
                                            op=OP.subtract)
                    nc.vector.tensor_scalar_add(h2[:], h2[:], mx[:])
                    nc.sync.dma_start(out_t[w * P:(w + 1) * P, :], h2[:])

    nc.compile()
    return nc


_CACHE = {}


def _get_built(key, R, W, K, wd0, wd1, bd, ln_trivial, b_zero):
    if key not in _CACHE:
        _CACHE[key] = _build(R, W, K, wd0, wd1, bd, ln_trivial, b_zero)
    return _CACHE[key]


def make_in_maps(inputs, prep):
    x = np.ascontiguousarray(np.asarray(inputs["x"], dtype=np.float32))
    n = x.shape[0]
    R = prep["R"]
    xp = np.zeros((NC * R, D), np.float32)
    xp[:n] = x
    lng = np.stack([np.asarray(inputs["ln1_g"], np.float32),
                    np.asarray(inputs["ln2_g"], np.float32)])
    lnb = np.stack([np.asarray(inputs["ln1_b"], np.float32),
                    np.asarray(inputs["ln2_b"], np.float32)])
    in_maps = []
    for c in range(NC):
        in_maps.append({
            "x": np.ascontiguousarray(xp[c * R:(c + 1) * R]),
            "W0": np.ascontiguousarray(np.asarray(inputs["W0"], np.float32)),
            "W1": np.ascontiguousarray(np.asarray(inputs["W1"], np.float32)),
            "b0": np.asarray(inputs["b0"], np.float32).reshape(1, D).copy(),
            "b1": np.asarray(inputs["b1"], np.float32).reshape(1, D).copy(),
            "idx16": prep["idx16"][c],
            "relc": prep["relc"][c], "hrev": prep["hrev"][c],
            "vmask": prep["vmask"][c], "mleft": prep["mleft"][c],
            "lng": np.ascontiguousarray(lng), "lnb": np.ascontiguousarray(lnb),
        })
    return in_maps


def _get_params(inputs):
    wd0 = float(np.asarray(inputs["drop_W"])[0, 0])
    wd1 = float(np.asarray(inputs["drop_W"])[0, 1])
    bd = float(np.asarray(inputs["drop_b"]).reshape(-1)[0])
    ln_trivial = all(
        np.all(np.asarray(inputs[k]) == v)
        for k, v in (("ln1_g", 1), ("ln2_g", 1), ("ln1_b", 0), ("ln2_b", 0)))
    b_zero = (np.all(np.asarray(inputs["b0"]) == 0)
              and np.all(np.asarray(inputs["b1"]) == 0))
    return wd0, wd1, bd, ln_trivial, b_zero


def kernel(**inputs):
    from concourse.bass_utils import run_bass_kernel_spmd

    row = np.asarray(inputs["row"])
    col = np.asarray(inputs["col"])
    n = np.asarray(inputs["x"]).shape[0]
    wd0, wd1, bd, ln_trivial, b_zero = _get_params(inputs)

    # drop gate is a no-op iff z = att*wd0 + att_rev*wd1 + bd > 0 whenever
    # att > 0 (given att, att_rev >= 0): wd0 > 0, wd1 >= 0, bd >= 0.
    if wd0 > 0 and wd1 >= 0 and bd >= 0 and n % NC == 0:
        prep = _prep_fast(row, col, n)
        return _kernel_fast(inputs, prep, ln_trivial, b_zero).astype(np.float32)

    prep = _preprocess(row, col, n)
    key = (n, prep["R"], prep["K"], wd0, wd1, bd, ln_trivial, b_zero)
    nc = _get_built(key, prep["R"], prep["W"], prep["K"], wd0, wd1, bd,
                    ln_trivial, b_zero)
    in_maps = make_in_maps(inputs, prep)
    res = run_bass_kernel_spmd(nc, in_maps, core_ids=list(range(NC)),
                               trace=bool(int(os.environ.get("GG_TRACE", "0"))))
    out = np.concatenate([r["out"] for r in res.results], axis=0)[:n]
    if os.environ.get("GG_RESULT_OBJ"):
        kernel._last_results = res
    return out.astype(np.float32)
